# revision 25
# baseline (speedup 1.0000x reference)
"""Multi-head attention (RoPE, causal) TRN2 Bass kernel, 8-way sharded.

Problem: B=4, S=1024, D=1024, H=16 heads of dim 64, fp32.
Sharding: batch (4) x head-half (2) -> 8 cores. Each core computes its
batch's attention output for its 8 heads and the partial output
projection (Wo row-block); the host sums the two half-head partials per
batch and adds the (bv @ Wo + bo) constant.

Per-core design (v2, bf16 datapath; ~131.5us cost-model vs 155.7us for
the f32r v1):
  - All activations/weights ship and compute in bf16 (1 cyc/row on PE at
    any N including the <256-column diagonal tiles, half the DMA/SBUF
    traffic); psum stays f32, output bf16 (summed in f32 on host).
  - Wq/Wk columns are permuted so each 128-row chunk holds 4 heads'
    even (or odd) RoPE coordinates; RoPE runs as 2 wide DVE
    scalar_tensor_tensor ops per psum pair ((e|o)+bias times the packed
    [cos|sin] table) and 2 combines split across DVE and the otherwise
    idle GPSIMD engine.
  - Per-head-contiguous [e;o] score layouts ("pair tiles") are built by
    [32,512] partition-offset copies (bf16 4x on DVE, rest on GPSIMD),
    letting each 128-chunk of transposed scores be 2 matmuls (K=64)
    instead of 4 (K=32) per head pair.
  - Causality: chunk skipping + q0 trimming; the partial diagonal block
    is zeroed after exp by a 2x bf16 DVE multiply with a 0/1 triangle.
  - exp() runs on ACT straight out of PSUM into bf16.
  - V gets a ones-column so softmax denominators fall out of the AV
    matmul (M=65); normalization: DVE reciprocal -> selector-matmul
    broadcast -> one DVE multiply per head into normalized bf16 ctx
    (evictions split ACT/DVE to sit in each engine's lulls).
  - Input DMAs are batched (few, large, rearranged descriptors) and
    ordered so the first rope group and qb0's AV unlock as early as
    possible; the final output tile is split across engines/DMAs to
    shorten the drain tail.
"""

import sys

sys.path.insert(0, "/opt/trn_rl_repo")

import numpy as np
import ml_dtypes

import concourse.bass as bass
import concourse.tile as tile
from concourse import bacc, mybir
from concourse.bass_utils import run_bass_kernel_spmd

P = 128
S = 1024
D = 1024
HD = 64
NH_LOCAL = 8  # heads per core
NB = 2  # S halves for projection psum
QB = 2  # q blocks of 512
KC = 8  # k chunks of 128
F32 = mybir.dt.float32
F32R = mybir.dt.float32r
BF16 = mybir.dt.bfloat16
F8 = mybir.dt.float8e4
DR = mybir.MatmulPerfMode.DoubleRow
# (x_half, w_half) product terms for the hi/lo fp8 split (lo*lo dropped)
FP8_TERMS = ((0, 0), (0, 1), (1, 0))
EXP = mybir.ActivationFunctionType.Exp
IDENT = mybir.ActivationFunctionType.Identity
MULT = mybir.AluOpType.mult
ADD = mybir.AluOpType.add
SUB = mybir.AluOpType.subtract

TRACE = False
LAST_RESULTS = None


def _build_core_program(causal=True):
    nc = bacc.Bacc(None, target_bir_lowering=False)

    # activations/weights ship as fp8 hi/lo plane pairs ([row, 2, col]):
    # hi = fp8(x), lo = fp8(x - hi). Projections run as DoubleRow fp8
    # matmuls over the 3 cross terms (hi*hi, hi*lo, lo*hi) at 0.5 cyc/row
    # -- 25% fewer PE cycles than bf16 at bf16-level precision.
    xqT = nc.declare_dram_parameter("xqT", [D, S, 2], F8, isOutput=False)
    xkT = nc.declare_dram_parameter("xkT", [D, S, 2], F8, isOutput=False)
    xvT = nc.declare_dram_parameter("xvT", [D, S, 2], F8, isOutput=False)
    wq = nc.declare_dram_parameter("wq", [D, 512, 2], F8, isOutput=False)
    wk = nc.declare_dram_parameter("wk", [D, 512, 2], F8, isOutput=False)
    wv = nc.declare_dram_parameter("wv", [D, 512, 2], F8, isOutput=False)
    wo = nc.declare_dram_parameter("wo", [512, D], BF16, isOutput=False)
    bqkp = nc.declare_dram_parameter("bqkp", [P, 8], F32, isOutput=False)
    # cos | sin rope tables side by side
    cs = nc.declare_dram_parameter("cs", [P, 2 * S], BF16, isOutput=False)
    # tri01[k, q] = 1 where q >= k else 0 (causal keep-mask for the
    # diagonal block), next to a PxP identity
    mi = nc.declare_dram_parameter("mi", [P, 2 * P], BF16, isOutput=False)
    selp = nc.declare_dram_parameter("selp", [2, P], BF16, isOutput=False)
    outp = nc.declare_dram_parameter("outp", [S, D], BF16, isOutput=True)

    with tile.TileContext(nc) as tc:
        with (
            tc.tile_pool(name="const", bufs=1) as cpool,
            tc.tile_pool(name="xt", bufs=6) as xtpool,
            tc.tile_pool(name="w", bufs=6) as wpool,
            tc.tile_pool(name="rot", bufs=1) as rotpool,
            tc.tile_pool(name="pair", bufs=1) as pairpool,
            tc.tile_pool(name="vsb", bufs=1) as vpool,
            tc.tile_pool(name="tmp", bufs=3) as tmppool,
            tc.tile_pool(name="es", bufs=16) as espool,
            tc.tile_pool(name="ctxe", bufs=4) as ctxepool,
            tc.tile_pool(name="ctx", bufs=3) as ctxpool,
            tc.tile_pool(name="osb", bufs=8) as opool,
            tc.tile_pool(name="dr", bufs=4) as drpool,
            tc.tile_pool(name="pj", bufs=1, space="PSUM") as pjpool,
            tc.tile_pool(name="sc", bufs=2, space="PSUM") as scpool,
            tc.tile_pool(name="cx", bufs=3, space="PSUM") as cxpool,
        ):
            # ---- constants ----
            cs_sb = cpool.tile([P, 2, S], BF16, tag="cs")
            cos_sb = cs_sb[:, 0, :]
            sin_sb = cs_sb[:, 1, :]
            mi_sb = cpool.tile([P, 2, P], BF16, tag="mi")
            tri_sb = mi_sb[:, 0, :]
            ident_sb = mi_sb[:, 1, :]
            bqk_sb = cpool.tile([P, 8], F32, tag="bqk")
            bq_sb = bqk_sb[:, 0:4]
            bk_sb = bqk_sb[:, 4:8]
            wo_sb = cpool.tile([P, 4, D], BF16, tag="wo")
            sel_sb = cpool.tile([2, P], BF16, tag="sel")
            ones8_sb = cpool.tile([P, NH_LOCAL], BF16, tag="ones8")

            warm = cpool.tile([1, 8], F32, tag="warm")
            nc.gpsimd.memset(warm[:], 0.0)
            nc.scalar.activation(warm[0:1, 4:8], warm[0:1, 0:4], EXP)

            def emit_consts():
                # only what the first rope group needs; the mask/selector
                # consts follow after k's x-loads
                nc.sync.dma_start(cs_sb[:], cs[:].rearrange("p (c s) -> p c s", c=2))
                nc.sync.dma_start(bqk_sb[:], bqkp[:])

            def emit_consts_late():
                nc.sync.dma_start(mi_sb[:], mi[:].rearrange("p (c s) -> p c s", c=2))
                nc.sync.dma_start(sel_sb[:], selp[:])
                # bf16 ones (Memset is avoided): derive from ident
                nc.vector.tensor_scalar(
                    ones8_sb[:], ident_sb[:, 0:NH_LOCAL], 0.0, 1.0, MULT, ADD
                )

            # ---- q/k projections + RoPE + pair-tile build ----
            # rot tiles per (name, G): e and o coordinate chunks [128, S]
            # pair tiles per (name, pair): [h0_e; h0_o; h1_e; h1_o] x [S]
            qpair = {}
            kpair = {}
            xt_cache = {}
            w_cache = {}
            for name in ("q", "k"):
                xt_cache[name] = [
                    xtpool.tile([P, KC // 2, S, 2], F8, tag="xt", name=f"xt_{name}{hf}")
                    for hf in range(2)
                ]
                w_cache[name] = [
                    wpool.tile([P, KC // 2, 512, 2], F8, tag="w", name=f"w_{name}{hf}")
                    for hf in range(2)
                ]

            def emit_w_chunk(name, w, hf, ks2):
                kg = hf * (KC // 2) + 2 * ks2
                nc.sync.dma_start(
                    w_cache[name][hf][:, 2 * ks2 : 2 * ks2 + 2, :, :],
                    w[kg * P : (kg + 2) * P, :, :].rearrange(
                        "(ko ki) f c -> ki ko (f c)", ki=P
                    ),
                )

            def emit_x_chunk(name, xT, nb, hf, ks2):
                kg = hf * (KC // 2) + 2 * ks2
                nc.sync.dma_start(
                    xt_cache[name][hf][
                        :, 2 * ks2 : 2 * ks2 + 2, nb * 512 : (nb + 1) * 512, :
                    ],
                    xT[
                        kg * P : (kg + 2) * P, nb * 512 : (nb + 1) * 512, :
                    ].rearrange("(ko ki) f c -> ki ko (f c)", ki=P),
                )

            def emit_w(name, w):
                for hf in range(2):
                    for ks2 in range(2):
                        emit_w_chunk(name, w, hf, ks2)

            def emit_x_half(name, xT, nb):
                for hf in range(2):
                    for ks2 in range(2):
                        emit_x_chunk(name, xT, nb, hf, ks2)

            def emit_wx_interleaved(name, w, xT, nb):
                # chunk-pair order matches the DR matmul consumption order so
                # the first matmul unblocks after two transfers
                for hf in range(2):
                    for ks2 in range(2):
                        emit_w_chunk(name, w, hf, ks2)
                        emit_x_chunk(name, xT, nb, hf, ks2)

            # nb0 halves + just-in-time consts feed the first rope groups;
            # nb1 halves follow after v's first column half
            emit_wx_interleaved("q", wq, xqT, 0)
            emit_consts()
            emit_wx_interleaved("k", wk, xkT, 0)

            # emit in an order that unlocks qb0 attention as early as
            # possible: (q,G0,nb0),(k,G0,nb0),(q,G1,nb0),(k,G1,nb0), then
            # the nb1 halves.
            def rope_group(name, G, nb):
                b_sb = bq_sb if name == "q" else bk_sb
                xt_h = xt_cache[name]
                w_h = w_cache[name]
                rots = rot_tiles[name]
                rot_e, rot_o = rots[G]
                ce, co = 2 * G, 2 * G + 1
                sl = slice(nb * 512, (nb + 1) * 512)
                ps_e_t = scpool.tile([P, 2, 512], F32, tag="sc", name=f"pse_{name}{G}{nb}")
                ps_e = ps_e_t[:, 0, :]
                ps_o = cxpool.tile([P, 512], F32, tag="cx", name=f"pso_{name}{G}{nb}")
                for ps, c in ((ps_e, ce), (ps_o, co)):
                    idx = 0
                    for hf in range(2):
                        for ks2 in range(2):
                            for xh, wh in FP8_TERMS:
                                nc.tensor.matmul(
                                    ps[:],
                                    w_h[hf][
                                        :, 2 * ks2 : 2 * ks2 + 2, c * P : (c + 1) * P, wh
                                    ],
                                    xt_h[hf][:, 2 * ks2 : 2 * ks2 + 2, sl, xh],
                                    start=(idx == 0),
                                    stop=(idx == 11),
                                    perf_mode=DR,
                                )
                                idx += 1
                # RoPE: rot_e = (e+be)c - (o+bo)s ; rot_o = (e+be)s + (o+bo)c
                # ACT (idle through the projection phase) evicts psum to bf16
                # with the per-partition bias add; the wide c|s products then
                # run as all-SBUF bf16 tensor_tensor on DVE at 4x
                t_eb = tmppool.tile([P, 512], BF16, tag="t1b")
                t_ob = tmppool.tile([P, 512], BF16, tag="t2b")
                nc.scalar.activation(t_eb[:], ps_e[:], IDENT, bias=b_sb[:, ce : ce + 1])
                nc.scalar.activation(t_ob[:], ps_o[:], IDENT, bias=b_sb[:, co : co + 1])
                t_e2 = tmppool.tile([P, 2, 512], BF16, tag="t1")
                t_o2 = tmppool.tile([P, 2, 512], BF16, tag="t2")
                nc.vector.tensor_tensor(
                    t_e2[:],
                    t_eb[:, None, :].to_broadcast((P, 2, 512)),
                    cs_sb[:, :, sl],
                    MULT,
                )
                nc.vector.tensor_tensor(
                    t_o2[:],
                    t_ob[:, None, :].to_broadcast((P, 2, 512)),
                    cs_sb[:, :, sl],
                    MULT,
                )
                # combines split across DVE and the (otherwise idle) GPSIMD
                # engine so both rot halves finish in parallel
                nc.vector.tensor_tensor(rot_e[:, sl], t_e2[:, 0, :], t_o2[:, 1, :], SUB)
                nc.gpsimd.tensor_tensor(rot_o[:, sl], t_e2[:, 1, :], t_o2[:, 0, :], ADD)
                # partition-move into per-head-contiguous pair tiles: bf16
                # SBUF->SBUF copies run at 4x on DVE; o-side copies go to the
                # GPSIMD engine to halve the chain latency
                pairs = qpair if name == "q" else kpair
                for hp in range(2):
                    pt = pairs[2 * G + hp]
                    for ii in range(2):
                        i = 2 * hp + ii
                        nc.vector.tensor_copy(
                            pt[64 * ii : 64 * ii + 32, sl],
                            rot_e[32 * i : 32 * i + 32, sl],
                        )
                        nc.gpsimd.tensor_copy(
                            pt[64 * ii + 32 : 64 * ii + 64, sl],
                            rot_o[32 * i : 32 * i + 32, sl],
                        )

            rot_tiles = {"q": {}, "k": {}}
            for name in ("q", "k"):
                for G in range(2):
                    rot_tiles[name][G] = (
                        rotpool.tile([P, S], BF16, tag=f"{name}re{G}", name=f"{name}re{G}"),
                        rotpool.tile([P, S], BF16, tag=f"{name}ro{G}", name=f"{name}ro{G}"),
                    )
            for pr in range(4):
                qpair[pr] = pairpool.tile([P, S], BF16, tag=f"qp{pr}", name=f"qp{pr}")
                kpair[pr] = pairpool.tile([P, S], BF16, tag=f"kp{pr}", name=f"kp{pr}")

            rope_group("q", 0, 0)
            rope_group("k", 0, 0)
            rope_group("q", 1, 0)
            rope_group("k", 1, 0)

            # ---- v projection (natural layout + ones column) ----
            # v x loaded by position-column halves: chunks 0..3 only read
            # columns [0:512]
            xt_h = []
            w_h = []
            for hf in range(2):
                xt_sb = xtpool.tile([P, KC // 2, S, 2], F8, tag="xt", name=f"xt_v{hf}")
                xt_h.append(xt_sb)
                w_sb = wpool.tile([P, KC // 2, 512, 2], F8, tag="w", name=f"w_v{hf}")
                kg = hf * (KC // 2)
                nc.sync.dma_start(
                    w_sb[:],
                    wv[kg * P : (kg + 4) * P, :, :].rearrange(
                        "(ko ki) f c -> ki ko (f c)", ki=P
                    ),
                )
                w_h.append(w_sb)
            emit_consts_late()

            def emit_xv_half(ch):
                for hf in range(2):
                    for ks2 in range(2):
                        kg = hf * (KC // 2) + 2 * ks2
                        nc.sync.dma_start(
                            xt_h[hf][
                                :, 2 * ks2 : 2 * ks2 + 2, ch * 512 : (ch + 1) * 512, :
                            ],
                            xvT[
                                kg * P : (kg + 2) * P, ch * 512 : (ch + 1) * 512, :
                            ].rearrange("(ko ki) f c -> ki ko (f c)", ki=P),
                        )

            emit_xv_half(0)
            emit_x_half("q", xqT, 1)
            emit_x_half("k", xkT, 1)
            emit_xv_half(1)
            nc.sync.dma_start(
                wo_sb[:], wo[:].rearrange("(ko ki) f -> ki ko f", ki=P)
            )

            v_sb = []
            v_order = [0, 1, 2, 3, 4, 5, 6, 7]
            v_tiles = {}
            for ki in v_order:
                vt = vpool.tile([P, NH_LOCAL, HD + 1], BF16, tag=f"v{ki}")
                v_tiles[ki] = vt
            def v_chunk(ki):
                vt = v_tiles[ki]
                ps_v = pjpool.tile([P, 512], F32, tag="pj", name=f"psv{ki}")
                idx = 0
                for hf in range(2):
                    for ks2 in range(2):
                        for xh, wh in FP8_TERMS:
                            nc.tensor.matmul(
                                ps_v[:],
                                xt_h[hf][
                                    :, 2 * ks2 : 2 * ks2 + 2, ki * P : (ki + 1) * P, xh
                                ],
                                w_h[hf][:, 2 * ks2 : 2 * ks2 + 2, :, wh],
                                start=(idx == 0),
                                stop=(idx == 11),
                                perf_mode=DR,
                            )
                            idx += 1
                nc.scalar.activation(
                    vt[:, :, 0:HD],
                    ps_v[:].rearrange("p (h d) -> p h d", h=NH_LOCAL),
                    mybir.ActivationFunctionType.Copy,
                    scale=1.0 / 512.0,
                )
                nc.vector.tensor_copy(vt[:, :, HD : HD + 1], ones8_sb[:, :, None])

            # ---- attention (emitted per (qb, pair)) ----
            # ctx pair tiles [128, 512] per (pair, qb): rows 0:64 head 2p,
            # rows 64:128 head 2p+1 (feature-transposed, normalized bf16)
            ctxp = {}
            norm_state = {}

            def attn_pair(qb, pair):
                kmax = 4 * qb + 4 if causal else KC
                qp = qpair[pair]
                kp = kpair[pair]
                ctx_ps = {}
                for ii in range(2):
                    ctx_ps[ii] = cxpool.tile(
                        [P, 512], F32, tag="cx", name=f"cx_{qb}_{2 * pair + ii}"
                    )
                # diagonal chunks (with the extra tri-multiply hop) first,
                # so the pair-closing AV is a short chain-free chunk
                if causal and qb == 1:
                    ki_order = [4, 5, 6, 7, 0, 1, 2, 3]
                else:
                    ki_order = list(range(kmax))
                for idx, ki in enumerate(ki_order):
                    ksl = slice(ki * P, (ki + 1) * P)
                    es2 = espool.tile([P, 2, 512], BF16, tag="es")
                    j = ki - 4 * qb if causal else -1
                    q0 = max(0, 128 * j)  # first live q col in this block
                    sc2 = scpool.tile([P, 2, 512], F32, tag="sc")
                    for ii in range(2):
                        nc.tensor.matmul(
                            sc2[:, ii, q0:],
                            kp[64 * ii : 64 * ii + 64, ksl],
                            qp[64 * ii : 64 * ii + 64, qb * 512 + q0 : (qb + 1) * 512],
                            start=True,
                            stop=True,
                        )
                    nc.scalar.activation(es2[:, :, q0:], sc2[:, :, q0:], EXP, scale=0.125)
                    if j >= 0:
                        # zero the above-diagonal block of the exp output
                        # (bf16 all-SBUF tensor_tensor runs at 2x on DVE)
                        nc.vector.tensor_tensor(
                            es2[:, :, 128 * j : 128 * (j + 1)],
                            es2[:, :, 128 * j : 128 * (j + 1)],
                            tri_sb[:, None, :].to_broadcast((P, 2, P)),
                            MULT,
                        )
                    for ii in range(2):
                        h = 2 * pair + ii
                        nc.tensor.matmul(
                            ctx_ps[ii][0 : HD + 1, q0:],
                            v_tiles[ki][:, h, :],
                            es2[:, ii, q0:],
                            start=(idx == 0),
                            stop=(idx == kmax - 1),
                        )
                # evict + reciprocal free the psum ring; the selector
                # broadcast + batched normalize are emitted LATER (see
                # attn_norm) so the next pair's scores/AV keep PE busy while
                # this pair's recip chain drains on ACT/DVE.
                cp = ctxpool.tile([P, 512], BF16, tag=f"cp{pair}", name=f"cp_{qb}_{pair}")
                ctxp[(pair, qb)] = cp
                drs = []
                for half in range(2):
                    if qb == 0 or pair == 3:
                        nc.scalar.copy(
                            cp[64 * half : 64 * half + 64, :], ctx_ps[half][0:HD, :]
                        )
                    else:
                        nc.vector.tensor_copy(
                            cp[64 * half : 64 * half + 64, :], ctx_ps[half][0:HD, :]
                        )
                    dr_t = drpool.tile(
                        [1, 512], BF16, tag="dr", name=f"dr_{qb}_{2 * pair + half}"
                    )
                    with nc.allow_low_precision(
                        reason="softmax denom reciprocal in bf16 (~4e-3 rel)"
                    ):
                        nc.vector.reciprocal(dr_t[0:1, :], ctx_ps[half][HD : HD + 1, :])
                    drs.append(dr_t)
                norm_state[(qb, pair)] = (cp, drs)

            def attn_norm(qb, pair):
                # broadcast each head's recips into its row-half of one shared
                # psum tile, then one batched in-place normalize multiply
                cp, drs = norm_state.pop((qb, pair))
                ps_b = pjpool.tile([P, 512], F32, tag="pj", name=f"psb_{qb}_{pair}")
                for half in range(2):
                    nc.tensor.matmul(
                        ps_b[64 * half : 64 * half + 64, :],
                        sel_sb[0:1, 0:HD],
                        drs[half][0:1, :],
                        start=True,
                        stop=True,
                    )
                nc.vector.tensor_tensor(cp[:], cp[:], ps_b[:], MULT)

            def out_proj(qb, qis=(0, 1, 2, 3)):
                for qi in qis:
                    o_sb = opool.tile([P, D], BF16, tag="o")
                    last = qb == 1 and qi == 3
                    q0g = (qb * 4 + qi) * P
                    for dh in range(2):
                        ps_o = pjpool.tile([P, 512], F32, tag="pj", name=f"po_{qb}_{qi}_{dh}")
                        for pidx in range(4):
                            nc.tensor.matmul(
                                ps_o[:],
                                ctxp[(pidx, qb)][:, qi * P : (qi + 1) * P],
                                wo_sb[:, pidx, dh * 512 : (dh + 1) * 512],
                                start=(pidx == 0),
                                stop=(pidx == 3),
                            )
                        if last and dh == 0:
                            # the very last output tile: split halves across
                            # DVE/ACT and DMA each half as soon as it lands
                            nc.vector.tensor_copy(
                                o_sb[:, dh * 512 : (dh + 1) * 512], ps_o[:]
                            )
                            nc.sync.dma_start(
                                outp[q0g : q0g + P, 0:512], o_sb[:, 0:512]
                            )
                        elif qb == 0:
                            # qb0's out tiles evict during qb1's exp-saturated
                            # window: keep them off ACT
                            nc.vector.tensor_copy(
                                o_sb[:, dh * 512 : (dh + 1) * 512], ps_o[:]
                            )
                        else:
                            nc.scalar.copy(o_sb[:, dh * 512 : (dh + 1) * 512], ps_o[:])
                    if last:
                        nc.sync.dma_start(
                            outp[q0g : q0g + P, 512:D], o_sb[:, 512:D]
                        )
                    else:
                        nc.sync.dma_start(outp[q0g : q0g + P, :], o_sb[:])

            # v chunks 0..3 unlock qb0's AV; emit attention pairs as soon as
            # their rope/pair/v dependencies exist so the scheduler can
            # overlap them with the remaining projections.
            v_chunk(0)
            v_chunk(1)
            rope_group("q", 0, 1)
            v_chunk(2)
            rope_group("k", 0, 1)
            v_chunk(3)
            attn_pair(0, 0)
            rope_group("q", 1, 1)
            attn_pair(0, 1)
            attn_norm(0, 0)
            v_chunk(4)
            attn_pair(0, 2)
            attn_norm(0, 1)
            rope_group("k", 1, 1)
            v_chunk(5)
            attn_pair(0, 3)
            attn_norm(0, 2)
            v_chunk(6)
            v_chunk(7)
            # qb0's output projection interleaves into qb1's exp-bound
            # attention stream as PE filler
            attn_pair(1, 0)
            attn_norm(0, 3)
            attn_norm(1, 0)
            attn_pair(1, 1)
            out_proj(0, qis=(0, 1))
            attn_pair(1, 2)
            attn_norm(1, 1)
            out_proj(0, qis=(2, 3))
            attn_pair(1, 3)
            attn_norm(1, 2)
            attn_norm(1, 3)
            out_proj(1)

    nc.compile()
    return nc


def _host_prep(query, key, value, Wq, bq, Wk, bk, Wv, bv, Wo, bo):
    """Build the 8 per-core input maps + the shared host-side constants."""
    B = query.shape[0]

    # RoPE tables (matches reference._rope_tables)
    inv_freq = (
        1.0 / (10000.0 ** (np.arange(0, HD, 2, dtype=np.float32) / HD))
    ).astype(np.float32)
    pos = np.arange(S, dtype=np.float32)
    ang = pos[:, None] * inv_freq[None, :]  # [S, 32]
    cos_t = np.cos(ang).astype(np.float32)  # [S, 32]
    sin_t = np.sin(ang).astype(np.float32)
    cosf = np.tile(cos_t.T, (4, 1)) / 512.0  # [128, S], undoes the x*8 w*64
    sinf = np.tile(sin_t.T, (4, 1)) / 512.0  # fp8-range pre-scales
    cs = np.concatenate([cosf, sinf], axis=1).astype(ml_dtypes.bfloat16)

    # additive causal mask for the diagonal block: out[k, q] needs
    # 0 where q >= k else -30; out = maskt.T so maskt[q, k]
    qq, kk = np.meshgrid(np.arange(P), np.arange(P), indexing="ij")
    # tri[k, q] = 1 where q >= k (keep), 0 above the causal diagonal
    tri01 = np.where(qq.T >= kk.T, 1.0, 0.0).astype(np.float32)
    ident = np.eye(P, dtype=np.float32)
    mi = np.concatenate([tri01, ident], axis=1).astype(ml_dtypes.bfloat16)
    selp = np.zeros((2, P), np.float32)
    selp[0, 0:64] = 1.0
    selp[1, 64:128] = 1.0
    selp = selp.astype(ml_dtypes.bfloat16)

    bf = ml_dtypes.bfloat16
    f8 = ml_dtypes.float8_e4m3

    # fp8 e4m3 normals start at 2^-6, so ship x pre-scaled by 8 and W by 64
    # (exact power-of-2 scales) to keep hi AND lo planes out of the
    # subnormal floor; the 1/512 descale folds into the cs table (q/k) and
    # the v-eviction Copy scale on device.
    BX, AW = 8.0, 64.0

    def split8(a, scale):
        # [R, C] float32 -> [R, C, 2] fp8 hi/lo planes
        a = a * np.float32(scale)
        hi = a.astype(f8)
        lo = (a - hi.astype(np.float32)).astype(f8)
        return np.ascontiguousarray(np.stack([hi, lo], axis=-1))

    xq8 = {b: split8(np.ascontiguousarray(query[b].T), BX) for b in range(4)}
    xk8 = {b: split8(np.ascontiguousarray(key[b].T), BX) for b in range(4)}
    xv8 = {b: split8(np.ascontiguousarray(value[b].T), BX) for b in range(4)}

    in_maps = []
    for c in range(8):
        b, g = c // 2, c % 2
        perm = np.concatenate(
            [
                (g * 8 + G * 4 + i) * HD + eo + 2 * np.arange(32)
                for G in range(2)
                for eo in range(2)
                for i in range(4)
            ]
        )
        wq_c = split8(Wq[:, perm], AW)
        bq_c = (bq[perm] * BX * AW).astype(np.float32).reshape(4, P).T
        wk_c = split8(Wk[:, perm], AW)
        bk_c = (bk[perm] * BX * AW).astype(np.float32).reshape(4, P).T
        bqk_c = np.concatenate([bq_c, bk_c], axis=1).astype(np.float32)
        wv_c = split8(Wv[:, g * 512 : (g + 1) * 512], AW)
        wo_c = Wo[g * 512 : (g + 1) * 512, :].astype(bf)
        in_maps.append(
            {
                "xqT": xq8[b],
                "xkT": xk8[b],
                "xvT": xv8[b],
                "wq": wq_c,
                "wk": wk_c,
                "wv": wv_c,
                "wo": np.ascontiguousarray(wo_c),
                "bqkp": np.ascontiguousarray(bqk_c),
                "cs": cs,
                "mi": mi,
                "selp": selp,
            }
        )
    extra = (bv.astype(np.float32) @ Wo.astype(np.float32) + bo).astype(np.float32)
    return in_maps, extra


_CACHED = {}


def kernel(query, key, value, mask, Wq, bq, Wk, bk, Wv, bv, Wo, bo):
    global LAST_RESULTS
    query = np.asarray(query, dtype=np.float32)
    key = np.asarray(key, dtype=np.float32)
    value = np.asarray(value, dtype=np.float32)
    Wq, bq = np.asarray(Wq, np.float32), np.asarray(bq, np.float32)
    Wk, bk = np.asarray(Wk, np.float32), np.asarray(bk, np.float32)
    Wv, bv = np.asarray(Wv, np.float32), np.asarray(bv, np.float32)
    Wo, bo = np.asarray(Wo, np.float32), np.asarray(bo, np.float32)

    assert query.shape == (4, S, D), f"kernel hardcodes B=4,S=1024,D=1024, got {query.shape}"
    m2 = np.asarray(mask).reshape(S, S)
    tril = np.tril(np.ones((S, S), m2.dtype))
    if np.array_equal(m2, tril):
        causal = True
    elif np.array_equal(m2, np.ones((S, S), m2.dtype)):
        causal = False
    else:
        raise NotImplementedError("kernel supports causal (tril) or all-ones masks")

    in_maps, extra = _host_prep(query, key, value, Wq, bq, Wk, bk, Wv, bv, Wo, bo)
    if causal not in _CACHED:
        _CACHED[causal] = _build_core_program(causal)
    res = run_bass_kernel_spmd(_CACHED[causal], in_maps, list(range(8)), trace=TRACE)
    LAST_RESULTS = res

    B = query.shape[0]
    out = np.empty((B, S, D), dtype=np.float32)
    for b in range(B):
        out[b] = (
            res.results[2 * b]["outp"].astype(np.float32)
            + res.results[2 * b + 1]["outp"].astype(np.float32)
            + extra
        )
    return out



# revision 26
# speedup vs baseline: 1.0852x; 1.0852x over previous
"""Multi-head attention (RoPE, causal) TRN2 Bass kernel, 8-way sharded.

Problem: B=4, S=1024, D=1024, H=16 heads of dim 64, fp32.
Sharding: batch (4) x head-half (2) -> 8 cores. Each core computes its
batch's attention output for its 8 heads and the partial output
projection (Wo row-block); the host sums the two half-head partials per
batch and adds the (bv @ Wo + bo) constant.

Per-core design (v2, bf16 datapath; ~131.5us cost-model vs 155.7us for
the f32r v1):
  - All activations/weights ship and compute in bf16 (1 cyc/row on PE at
    any N including the <256-column diagonal tiles, half the DMA/SBUF
    traffic); psum stays f32, output bf16 (summed in f32 on host).
  - Wq/Wk columns are permuted so each 128-row chunk holds 4 heads'
    even (or odd) RoPE coordinates; RoPE runs as 2 wide DVE
    scalar_tensor_tensor ops per psum pair ((e|o)+bias times the packed
    [cos|sin] table) and 2 combines split across DVE and the otherwise
    idle GPSIMD engine.
  - Per-head-contiguous [e;o] score layouts ("pair tiles") are built by
    [32,512] partition-offset copies (bf16 4x on DVE, rest on GPSIMD),
    letting each 128-chunk of transposed scores be 2 matmuls (K=64)
    instead of 4 (K=32) per head pair.
  - Causality: chunk skipping + q0 trimming; the partial diagonal block
    is zeroed after exp by a 2x bf16 DVE multiply with a 0/1 triangle.
  - exp() runs on ACT straight out of PSUM into bf16.
  - V gets a ones-column so softmax denominators fall out of the AV
    matmul (M=65); normalization: DVE reciprocal -> selector-matmul
    broadcast -> one DVE multiply per head into normalized bf16 ctx
    (evictions split ACT/DVE to sit in each engine's lulls).
  - Input DMAs are batched (few, large, rearranged descriptors) and
    ordered so the first rope group and qb0's AV unlock as early as
    possible; the final output tile is split across engines/DMAs to
    shorten the drain tail.
"""

import sys

sys.path.insert(0, "/opt/trn_rl_repo")

import numpy as np
import ml_dtypes

import concourse.bass as bass
import concourse.tile as tile
from concourse import bacc, mybir
from concourse.bass_utils import run_bass_kernel_spmd

P = 128
S = 1024
D = 1024
HD = 64
NH_LOCAL = 8  # heads per core
NB = 2  # S halves for projection psum
QB = 2  # q blocks of 512
KC = 8  # k chunks of 128
F32 = mybir.dt.float32
F32R = mybir.dt.float32r
BF16 = mybir.dt.bfloat16
F8 = mybir.dt.float8e4
DR = mybir.MatmulPerfMode.DoubleRow
# (x_half, w_half) product terms for the hi/lo fp8 split (lo*lo dropped)
FP8_TERMS = ((0, 0), (0, 1), (1, 0))
EXP = mybir.ActivationFunctionType.Exp
IDENT = mybir.ActivationFunctionType.Identity
MULT = mybir.AluOpType.mult
ADD = mybir.AluOpType.add
SUB = mybir.AluOpType.subtract

TRACE = False
LAST_RESULTS = None


def _build_core_program(causal=True):
    nc = bacc.Bacc(None, target_bir_lowering=False)

    # activations/weights ship as fp8 hi/lo plane pairs ([row, 2, col]):
    # hi = fp8(x), lo = fp8(x - hi). Projections run as DoubleRow fp8
    # matmuls over the 3 cross terms (hi*hi, hi*lo, lo*hi) at 0.5 cyc/row
    # -- 25% fewer PE cycles than bf16 at bf16-level precision.
    xqT = nc.declare_dram_parameter("xqT", [D, S, 2], F8, isOutput=False)
    xkT = nc.declare_dram_parameter("xkT", [D, S, 2], F8, isOutput=False)
    xvT = nc.declare_dram_parameter("xvT", [D, S, 2], F8, isOutput=False)
    wq = nc.declare_dram_parameter("wq", [D, 512, 2], F8, isOutput=False)
    wk = nc.declare_dram_parameter("wk", [D, 512, 2], F8, isOutput=False)
    wv = nc.declare_dram_parameter("wv", [D, 512, 2], F8, isOutput=False)
    wo = nc.declare_dram_parameter("wo", [512, D], BF16, isOutput=False)
    bqkp = nc.declare_dram_parameter("bqkp", [P, 8], F32, isOutput=False)
    # cos | sin rope tables side by side
    cs = nc.declare_dram_parameter("cs", [P, 2 * S], BF16, isOutput=False)
    # tri01[k, q] = 1 where q >= k else 0 (causal keep-mask for the
    # diagonal block), next to a PxP identity
    mi = nc.declare_dram_parameter("mi", [P, 2 * P], BF16, isOutput=False)
    selp = nc.declare_dram_parameter("selp", [2, P], BF16, isOutput=False)
    outp = nc.declare_dram_parameter("outp", [S, D], BF16, isOutput=True)

    with tile.TileContext(nc) as tc:
        with (
            tc.tile_pool(name="const", bufs=1) as cpool,
            tc.tile_pool(name="xt", bufs=6) as xtpool,
            tc.tile_pool(name="w", bufs=6) as wpool,
            tc.tile_pool(name="rot", bufs=1) as rotpool,
            tc.tile_pool(name="pair", bufs=1) as pairpool,
            tc.tile_pool(name="vsb", bufs=1) as vpool,
            tc.tile_pool(name="tmp", bufs=3) as tmppool,
            tc.tile_pool(name="es", bufs=16) as espool,
            tc.tile_pool(name="ctxe", bufs=4) as ctxepool,
            tc.tile_pool(name="ctx", bufs=3) as ctxpool,
            tc.tile_pool(name="osb", bufs=8) as opool,
            tc.tile_pool(name="dr", bufs=4) as drpool,
            tc.tile_pool(name="pj", bufs=2, space="PSUM") as pjpool,
            tc.tile_pool(name="sc", bufs=2, space="PSUM") as scpool,
            tc.tile_pool(name="cx", bufs=2, space="PSUM") as cxpool,
        ):
            # ---- constants ----
            cs_sb = cpool.tile([P, 2, S], BF16, tag="cs")
            cos_sb = cs_sb[:, 0, :]
            sin_sb = cs_sb[:, 1, :]
            mi_sb = cpool.tile([P, 2, P], BF16, tag="mi")
            tri_sb = mi_sb[:, 0, :]
            ident_sb = mi_sb[:, 1, :]
            bqk_sb = cpool.tile([P, 8], F32, tag="bqk")
            bq_sb = bqk_sb[:, 0:4]
            bk_sb = bqk_sb[:, 4:8]
            wo_sb = cpool.tile([P, 4, D], BF16, tag="wo")
            sel_sb = cpool.tile([2, P], BF16, tag="sel")
            ones8_sb = cpool.tile([P, NH_LOCAL], BF16, tag="ones8")

            warm = cpool.tile([1, 8], F32, tag="warm")
            nc.gpsimd.memset(warm[:], 0.0)
            nc.scalar.activation(warm[0:1, 4:8], warm[0:1, 0:4], EXP)

            def emit_consts():
                # only what the first rope group needs; the mask/selector
                # consts follow after k's x-loads
                nc.sync.dma_start(cs_sb[:], cs[:].rearrange("p (c s) -> p c s", c=2))
                nc.sync.dma_start(bqk_sb[:], bqkp[:])

            def emit_consts_late():
                nc.sync.dma_start(mi_sb[:], mi[:].rearrange("p (c s) -> p c s", c=2))
                nc.sync.dma_start(sel_sb[:], selp[:])
                # bf16 ones (Memset is avoided): derive from ident
                nc.vector.tensor_scalar(
                    ones8_sb[:], ident_sb[:, 0:NH_LOCAL], 0.0, 1.0, MULT, ADD
                )

            # ---- q/k projections + RoPE + pair-tile build ----
            # rot tiles per (name, G): e and o coordinate chunks [128, S]
            # pair tiles per (name, pair): [h0_e; h0_o; h1_e; h1_o] x [S]
            qpair = {}
            kpair = {}
            xt_cache = {}
            w_cache = {}
            for name in ("q", "k"):
                xt_cache[name] = [
                    xtpool.tile([P, KC // 2, S, 2], F8, tag="xt", name=f"xt_{name}{hf}")
                    for hf in range(2)
                ]
                w_cache[name] = [
                    wpool.tile([P, KC // 2, 512, 2], F8, tag="w", name=f"w_{name}{hf}")
                    for hf in range(2)
                ]

            def emit_w_chunk(name, w, hf, ks2):
                kg = hf * (KC // 2) + 2 * ks2
                nc.sync.dma_start(
                    w_cache[name][hf][:, 2 * ks2 : 2 * ks2 + 2, :, :],
                    w[kg * P : (kg + 2) * P, :, :].rearrange(
                        "(ko ki) f c -> ki ko (f c)", ki=P
                    ),
                )

            def emit_x_chunk(name, xT, nb, hf, ks2):
                kg = hf * (KC // 2) + 2 * ks2
                nc.sync.dma_start(
                    xt_cache[name][hf][
                        :, 2 * ks2 : 2 * ks2 + 2, nb * 512 : (nb + 1) * 512, :
                    ],
                    xT[
                        kg * P : (kg + 2) * P, nb * 512 : (nb + 1) * 512, :
                    ].rearrange("(ko ki) f c -> ki ko (f c)", ki=P),
                )

            def emit_w(name, w):
                for hf in range(2):
                    for ks2 in range(2):
                        emit_w_chunk(name, w, hf, ks2)

            def emit_x_half(name, xT, nb):
                for hf in range(2):
                    for ks2 in range(2):
                        emit_x_chunk(name, xT, nb, hf, ks2)

            def emit_wx_interleaved(name, w, xT, nb):
                # chunk-pair order matches the DR matmul consumption order so
                # the first matmul unblocks after two transfers
                for hf in range(2):
                    for ks2 in range(2):
                        emit_w_chunk(name, w, hf, ks2)
                        emit_x_chunk(name, xT, nb, hf, ks2)

            # nb0 halves + just-in-time consts feed the first rope groups;
            # nb1 halves follow after v's first column half
            emit_wx_interleaved("q", wq, xqT, 0)
            emit_consts()
            emit_wx_interleaved("k", wk, xkT, 0)

            # emit in an order that unlocks qb0 attention as early as
            # possible: (q,G0,nb0),(k,G0,nb0),(q,G1,nb0),(k,G1,nb0), then
            # the nb1 halves.
            def rope_group(name, G, nb):
                b_sb = bq_sb if name == "q" else bk_sb
                xt_h = xt_cache[name]
                w_h = w_cache[name]
                rots = rot_tiles[name]
                rot_e, rot_o = rots[G]
                ce, co = 2 * G, 2 * G + 1
                sl = slice(nb * 512, (nb + 1) * 512)
                ps_e_t = scpool.tile([P, 2, 512], F32, tag="sc", name=f"pse_{name}{G}{nb}")
                ps_e = ps_e_t[:, 0, :]
                ps_o = cxpool.tile([P, 512], F32, tag="cx", name=f"pso_{name}{G}{nb}")
                for ps, c in ((ps_e, ce), (ps_o, co)):
                    idx = 0
                    for hf in range(2):
                        for ks2 in range(2):
                            for xh, wh in FP8_TERMS:
                                nc.tensor.matmul(
                                    ps[:],
                                    w_h[hf][
                                        :, 2 * ks2 : 2 * ks2 + 2, c * P : (c + 1) * P, wh
                                    ],
                                    xt_h[hf][:, 2 * ks2 : 2 * ks2 + 2, sl, xh],
                                    start=(idx == 0),
                                    stop=(idx == 11),
                                    perf_mode=DR,
                                )
                                idx += 1
                # RoPE: rot_e = (e+be)c - (o+bo)s ; rot_o = (e+be)s + (o+bo)c
                # ACT (idle through the projection phase) evicts psum to bf16
                # with the per-partition bias add; the wide c|s products then
                # run as all-SBUF bf16 tensor_tensor on DVE at 4x
                t_eb = tmppool.tile([P, 512], BF16, tag="t1b")
                t_ob = tmppool.tile([P, 512], BF16, tag="t2b")
                nc.scalar.activation(t_eb[:], ps_e[:], IDENT, bias=b_sb[:, ce : ce + 1])
                nc.scalar.activation(t_ob[:], ps_o[:], IDENT, bias=b_sb[:, co : co + 1])
                t_e2 = tmppool.tile([P, 2, 512], BF16, tag="t1")
                t_o2 = tmppool.tile([P, 2, 512], BF16, tag="t2")
                nc.vector.tensor_tensor(
                    t_e2[:],
                    t_eb[:, None, :].to_broadcast((P, 2, 512)),
                    cs_sb[:, :, sl],
                    MULT,
                )
                nc.vector.tensor_tensor(
                    t_o2[:],
                    t_ob[:, None, :].to_broadcast((P, 2, 512)),
                    cs_sb[:, :, sl],
                    MULT,
                )
                # combines split across DVE and the (otherwise idle) GPSIMD
                # engine so both rot halves finish in parallel
                nc.vector.tensor_tensor(rot_e[:, sl], t_e2[:, 0, :], t_o2[:, 1, :], SUB)
                nc.gpsimd.tensor_tensor(rot_o[:, sl], t_e2[:, 1, :], t_o2[:, 0, :], ADD)
                # partition-move into per-head-contiguous pair tiles: bf16
                # SBUF->SBUF copies run at 4x on DVE; o-side copies go to the
                # GPSIMD engine to halve the chain latency
                pairs = qpair if name == "q" else kpair
                for hp in range(2):
                    pt = pairs[2 * G + hp]
                    for ii in range(2):
                        i = 2 * hp + ii
                        nc.vector.tensor_copy(
                            pt[64 * ii : 64 * ii + 32, sl],
                            rot_e[32 * i : 32 * i + 32, sl],
                        )
                        nc.gpsimd.tensor_copy(
                            pt[64 * ii + 32 : 64 * ii + 64, sl],
                            rot_o[32 * i : 32 * i + 32, sl],
                        )

            rot_tiles = {"q": {}, "k": {}}
            for name in ("q", "k"):
                for G in range(2):
                    rot_tiles[name][G] = (
                        rotpool.tile([P, S], BF16, tag=f"{name}re{G}", name=f"{name}re{G}"),
                        rotpool.tile([P, S], BF16, tag=f"{name}ro{G}", name=f"{name}ro{G}"),
                    )
            for pr in range(4):
                qpair[pr] = pairpool.tile([P, S], BF16, tag=f"qp{pr}", name=f"qp{pr}")
                kpair[pr] = pairpool.tile([P, S], BF16, tag=f"kp{pr}", name=f"kp{pr}")

            rope_group("q", 0, 0)
            rope_group("k", 0, 0)
            rope_group("q", 1, 0)
            rope_group("k", 1, 0)

            # ---- v projection (natural layout + ones column) ----
            # v x loaded by position-column halves: chunks 0..3 only read
            # columns [0:512]
            xt_h = []
            w_h = []
            for hf in range(2):
                xt_sb = xtpool.tile([P, KC // 2, S, 2], F8, tag="xt", name=f"xt_v{hf}")
                xt_h.append(xt_sb)
                w_sb = wpool.tile([P, KC // 2, 512, 2], F8, tag="w", name=f"w_v{hf}")
                kg = hf * (KC // 2)
                nc.sync.dma_start(
                    w_sb[:],
                    wv[kg * P : (kg + 4) * P, :, :].rearrange(
                        "(ko ki) f c -> ki ko (f c)", ki=P
                    ),
                )
                w_h.append(w_sb)
            emit_consts_late()

            def emit_xv_half(ch):
                for hf in range(2):
                    for ks2 in range(2):
                        kg = hf * (KC // 2) + 2 * ks2
                        nc.sync.dma_start(
                            xt_h[hf][
                                :, 2 * ks2 : 2 * ks2 + 2, ch * 512 : (ch + 1) * 512, :
                            ],
                            xvT[
                                kg * P : (kg + 2) * P, ch * 512 : (ch + 1) * 512, :
                            ].rearrange("(ko ki) f c -> ki ko (f c)", ki=P),
                        )

            emit_xv_half(0)
            emit_x_half("q", xqT, 1)
            emit_x_half("k", xkT, 1)
            emit_xv_half(1)
            nc.sync.dma_start(
                wo_sb[:], wo[:].rearrange("(ko ki) f -> ki ko f", ki=P)
            )

            v_sb = []
            v_order = [0, 1, 2, 3, 4, 5, 6, 7]
            v_tiles = {}
            for ki in v_order:
                vt = vpool.tile([P, NH_LOCAL, HD + 1], BF16, tag=f"v{ki}")
                v_tiles[ki] = vt
            def v_chunk(ki):
                vt = v_tiles[ki]
                ps_v = pjpool.tile([P, 512], F32, tag="pj", name=f"psv{ki}")
                idx = 0
                for hf in range(2):
                    for ks2 in range(2):
                        for xh, wh in FP8_TERMS:
                            nc.tensor.matmul(
                                ps_v[:],
                                xt_h[hf][
                                    :, 2 * ks2 : 2 * ks2 + 2, ki * P : (ki + 1) * P, xh
                                ],
                                w_h[hf][:, 2 * ks2 : 2 * ks2 + 2, :, wh],
                                start=(idx == 0),
                                stop=(idx == 11),
                                perf_mode=DR,
                            )
                            idx += 1
                nc.scalar.activation(
                    vt[:, :, 0:HD],
                    ps_v[:].rearrange("p (h d) -> p h d", h=NH_LOCAL),
                    mybir.ActivationFunctionType.Copy,
                    scale=1.0 / 512.0,
                )
                nc.vector.tensor_copy(vt[:, :, HD : HD + 1], ones8_sb[:, :, None])

            # ---- attention (emitted per (qb, pair)) ----
            # ctx pair tiles [128, 512] per (pair, qb): rows 0:64 head 2p,
            # rows 64:128 head 2p+1 (feature-transposed, normalized bf16)
            ctxp = {}
            norm_state = {}

            def attn_pair(qb, pair):
                kmax = 4 * qb + 4 if causal else KC
                qp = qpair[pair]
                kp = kpair[pair]
                ctx_ps = {}
                for ii in range(2):
                    ctx_ps[ii] = cxpool.tile(
                        [P, 512], F32, tag="cx", name=f"cx_{qb}_{2 * pair + ii}"
                    )
                # diagonal chunks (with the extra tri-multiply hop) first,
                # so the pair-closing AV is a short chain-free chunk
                if causal and qb == 1:
                    ki_order = [4, 5, 6, 7, 0, 1, 2, 3]
                else:
                    ki_order = list(range(kmax))
                def emit_scores(ki):
                    ksl = slice(ki * P, (ki + 1) * P)
                    es2 = espool.tile([P, 2, 512], BF16, tag="es")
                    j = ki - 4 * qb if causal else -1
                    q0 = max(0, 128 * j)  # first live q col in this block
                    sc2 = scpool.tile([P, 2, 512], F32, tag="sc")
                    for ii in range(2):
                        nc.tensor.matmul(
                            sc2[:, ii, q0:],
                            kp[64 * ii : 64 * ii + 64, ksl],
                            qp[64 * ii : 64 * ii + 64, qb * 512 + q0 : (qb + 1) * 512],
                            start=True,
                            stop=True,
                        )
                    nc.scalar.activation(es2[:, :, q0:], sc2[:, :, q0:], EXP, scale=0.125)
                    if j >= 0:
                        # zero the above-diagonal block of the exp output
                        # (bf16 all-SBUF tensor_tensor runs at 2x on DVE)
                        nc.vector.tensor_tensor(
                            es2[:, :, 128 * j : 128 * (j + 1)],
                            es2[:, :, 128 * j : 128 * (j + 1)],
                            tri_sb[:, None, :].to_broadcast((P, 2, P)),
                            MULT,
                        )
                    return es2, q0

                def emit_av(idx, ki, es2, q0):
                    for ii in range(2):
                        h = 2 * pair + ii
                        nc.tensor.matmul(
                            ctx_ps[ii][0 : HD + 1, q0:],
                            v_tiles[ki][:, h, :],
                            es2[:, ii, q0:],
                            start=(idx == 0),
                            stop=(idx == kmax - 1),
                        )

                # AV trails the scores by one chunk so the pair's first AV
                # (which waits on the cx-ring free) never heads the PE queue
                pend = None
                for idx, ki in enumerate(ki_order):
                    es2, q0 = emit_scores(ki)
                    if pend is not None:
                        emit_av(*pend)
                    pend = (idx, ki, es2, q0)
                emit_av(*pend)
                # evict + reciprocal free the psum ring; the selector
                # broadcast + batched normalize are emitted LATER (see
                # attn_norm) so the next pair's scores/AV keep PE busy while
                # this pair's recip chain drains on ACT/DVE.
                cp = ctxpool.tile([P, 512], BF16, tag=f"cp{pair}", name=f"cp_{qb}_{pair}")
                ctxp[(pair, qb)] = cp
                drs = []
                for half in range(2):
                    if qb == 0 or pair == 3:
                        nc.scalar.copy(
                            cp[64 * half : 64 * half + 64, :], ctx_ps[half][0:HD, :]
                        )
                    else:
                        nc.vector.tensor_copy(
                            cp[64 * half : 64 * half + 64, :], ctx_ps[half][0:HD, :]
                        )
                    dr_t = drpool.tile(
                        [1, 512], BF16, tag="dr", name=f"dr_{qb}_{2 * pair + half}"
                    )
                    with nc.allow_low_precision(
                        reason="softmax denom reciprocal in bf16 (~4e-3 rel)"
                    ):
                        nc.vector.reciprocal(dr_t[0:1, :], ctx_ps[half][HD : HD + 1, :])
                    drs.append(dr_t)
                norm_state[(qb, pair)] = (cp, drs)

            def attn_norm(qb, pair):
                # broadcast each head's recips into its row-half of one shared
                # psum tile, then one batched in-place normalize multiply
                cp, drs = norm_state.pop((qb, pair))
                ps_b = pjpool.tile([P, 512], F32, tag="pj", name=f"psb_{qb}_{pair}")
                for half in range(2):
                    nc.tensor.matmul(
                        ps_b[64 * half : 64 * half + 64, :],
                        sel_sb[0:1, 0:HD],
                        drs[half][0:1, :],
                        start=True,
                        stop=True,
                    )
                nc.vector.tensor_tensor(cp[:], cp[:], ps_b[:], MULT)

            def out_proj(qb, qis=(0, 1, 2, 3)):
                for qi in qis:
                    o_sb = opool.tile([P, D], BF16, tag="o")
                    last = qb == 1 and qi == 3
                    q0g = (qb * 4 + qi) * P
                    for dh in range(2):
                        ps_o = pjpool.tile([P, 512], F32, tag="pj", name=f"po_{qb}_{qi}_{dh}")
                        for pidx in range(4):
                            nc.tensor.matmul(
                                ps_o[:],
                                ctxp[(pidx, qb)][:, qi * P : (qi + 1) * P],
                                wo_sb[:, pidx, dh * 512 : (dh + 1) * 512],
                                start=(pidx == 0),
                                stop=(pidx == 3),
                            )
                        if last and dh == 0:
                            # the very last output tile: split halves across
                            # DVE/ACT and DMA each half as soon as it lands
                            nc.vector.tensor_copy(
                                o_sb[:, dh * 512 : (dh + 1) * 512], ps_o[:]
                            )
                            nc.sync.dma_start(
                                outp[q0g : q0g + P, 0:512], o_sb[:, 0:512]
                            )
                        elif qb == 0:
                            # qb0's out tiles evict during qb1's exp-saturated
                            # window: keep them off ACT
                            nc.vector.tensor_copy(
                                o_sb[:, dh * 512 : (dh + 1) * 512], ps_o[:]
                            )
                        else:
                            nc.scalar.copy(o_sb[:, dh * 512 : (dh + 1) * 512], ps_o[:])
                    if last:
                        nc.sync.dma_start(
                            outp[q0g : q0g + P, 512:D], o_sb[:, 512:D]
                        )
                    else:
                        nc.sync.dma_start(outp[q0g : q0g + P, :], o_sb[:])

            # v chunks 0..3 unlock qb0's AV; emit attention pairs as soon as
            # their rope/pair/v dependencies exist so the scheduler can
            # overlap them with the remaining projections.
            v_chunk(0)
            v_chunk(1)
            rope_group("q", 0, 1)
            v_chunk(2)
            rope_group("k", 0, 1)
            v_chunk(3)
            attn_pair(0, 0)
            rope_group("q", 1, 1)
            attn_pair(0, 1)
            attn_norm(0, 0)
            v_chunk(4)
            attn_pair(0, 2)
            attn_norm(0, 1)
            rope_group("k", 1, 1)
            v_chunk(5)
            attn_pair(0, 3)
            attn_norm(0, 2)
            v_chunk(6)
            v_chunk(7)
            # qb0's output projection interleaves into qb1's exp-bound
            # attention stream as PE filler
            attn_pair(1, 0)
            attn_norm(0, 3)
            attn_norm(1, 0)
            attn_pair(1, 1)
            out_proj(0, qis=(0, 1))
            attn_pair(1, 2)
            attn_norm(1, 1)
            out_proj(0, qis=(2, 3))
            attn_pair(1, 3)
            attn_norm(1, 2)
            attn_norm(1, 3)
            out_proj(1)

    nc.compile()
    return nc


def _host_prep(query, key, value, Wq, bq, Wk, bk, Wv, bv, Wo, bo):
    """Build the 8 per-core input maps + the shared host-side constants."""
    B = query.shape[0]

    # RoPE tables (matches reference._rope_tables)
    inv_freq = (
        1.0 / (10000.0 ** (np.arange(0, HD, 2, dtype=np.float32) / HD))
    ).astype(np.float32)
    pos = np.arange(S, dtype=np.float32)
    ang = pos[:, None] * inv_freq[None, :]  # [S, 32]
    cos_t = np.cos(ang).astype(np.float32)  # [S, 32]
    sin_t = np.sin(ang).astype(np.float32)
    cosf = np.tile(cos_t.T, (4, 1)) / 512.0  # [128, S], undoes the x*8 w*64
    sinf = np.tile(sin_t.T, (4, 1)) / 512.0  # fp8-range pre-scales
    cs = np.concatenate([cosf, sinf], axis=1).astype(ml_dtypes.bfloat16)

    # additive causal mask for the diagonal block: out[k, q] needs
    # 0 where q >= k else -30; out = maskt.T so maskt[q, k]
    qq, kk = np.meshgrid(np.arange(P), np.arange(P), indexing="ij")
    # tri[k, q] = 1 where q >= k (keep), 0 above the causal diagonal
    tri01 = np.where(qq.T >= kk.T, 1.0, 0.0).astype(np.float32)
    ident = np.eye(P, dtype=np.float32)
    mi = np.concatenate([tri01, ident], axis=1).astype(ml_dtypes.bfloat16)
    selp = np.zeros((2, P), np.float32)
    selp[0, 0:64] = 1.0
    selp[1, 64:128] = 1.0
    selp = selp.astype(ml_dtypes.bfloat16)

    bf = ml_dtypes.bfloat16
    f8 = ml_dtypes.float8_e4m3

    # fp8 e4m3 normals start at 2^-6, so ship x pre-scaled by 8 and W by 64
    # (exact power-of-2 scales) to keep hi AND lo planes out of the
    # subnormal floor; the 1/512 descale folds into the cs table (q/k) and
    # the v-eviction Copy scale on device.
    BX, AW = 8.0, 64.0

    def split8(a, scale):
        # [R, C] float32 -> [R, C, 2] fp8 hi/lo planes
        a = a * np.float32(scale)
        hi = a.astype(f8)
        lo = (a - hi.astype(np.float32)).astype(f8)
        return np.ascontiguousarray(np.stack([hi, lo], axis=-1))

    xq8 = {b: split8(np.ascontiguousarray(query[b].T), BX) for b in range(4)}
    xk8 = {b: split8(np.ascontiguousarray(key[b].T), BX) for b in range(4)}
    xv8 = {b: split8(np.ascontiguousarray(value[b].T), BX) for b in range(4)}

    in_maps = []
    for c in range(8):
        b, g = c // 2, c % 2
        perm = np.concatenate(
            [
                (g * 8 + G * 4 + i) * HD + eo + 2 * np.arange(32)
                for G in range(2)
                for eo in range(2)
                for i in range(4)
            ]
        )
        wq_c = split8(Wq[:, perm], AW)
        bq_c = (bq[perm] * BX * AW).astype(np.float32).reshape(4, P).T
        wk_c = split8(Wk[:, perm], AW)
        bk_c = (bk[perm] * BX * AW).astype(np.float32).reshape(4, P).T
        bqk_c = np.concatenate([bq_c, bk_c], axis=1).astype(np.float32)
        wv_c = split8(Wv[:, g * 512 : (g + 1) * 512], AW)
        wo_c = Wo[g * 512 : (g + 1) * 512, :].astype(bf)
        in_maps.append(
            {
                "xqT": xq8[b],
                "xkT": xk8[b],
                "xvT": xv8[b],
                "wq": wq_c,
                "wk": wk_c,
                "wv": wv_c,
                "wo": np.ascontiguousarray(wo_c),
                "bqkp": np.ascontiguousarray(bqk_c),
                "cs": cs,
                "mi": mi,
                "selp": selp,
            }
        )
    extra = (bv.astype(np.float32) @ Wo.astype(np.float32) + bo).astype(np.float32)
    return in_maps, extra


_CACHED = {}


def kernel(query, key, value, mask, Wq, bq, Wk, bk, Wv, bv, Wo, bo):
    global LAST_RESULTS
    query = np.asarray(query, dtype=np.float32)
    key = np.asarray(key, dtype=np.float32)
    value = np.asarray(value, dtype=np.float32)
    Wq, bq = np.asarray(Wq, np.float32), np.asarray(bq, np.float32)
    Wk, bk = np.asarray(Wk, np.float32), np.asarray(bk, np.float32)
    Wv, bv = np.asarray(Wv, np.float32), np.asarray(bv, np.float32)
    Wo, bo = np.asarray(Wo, np.float32), np.asarray(bo, np.float32)

    assert query.shape == (4, S, D), f"kernel hardcodes B=4,S=1024,D=1024, got {query.shape}"
    m2 = np.asarray(mask).reshape(S, S)
    tril = np.tril(np.ones((S, S), m2.dtype))
    if np.array_equal(m2, tril):
        causal = True
    elif np.array_equal(m2, np.ones((S, S), m2.dtype)):
        causal = False
    else:
        raise NotImplementedError("kernel supports causal (tril) or all-ones masks")

    in_maps, extra = _host_prep(query, key, value, Wq, bq, Wk, bk, Wv, bv, Wo, bo)
    if causal not in _CACHED:
        _CACHED[causal] = _build_core_program(causal)
    res = run_bass_kernel_spmd(_CACHED[causal], in_maps, list(range(8)), trace=TRACE)
    LAST_RESULTS = res

    B = query.shape[0]
    out = np.empty((B, S, D), dtype=np.float32)
    for b in range(B):
        out[b] = (
            res.results[2 * b]["outp"].astype(np.float32)
            + res.results[2 * b + 1]["outp"].astype(np.float32)
            + extra
        )
    return out



# revision 27
# speedup vs baseline: 1.1184x; 1.0305x over previous
"""Multi-head attention (RoPE, causal) TRN2 Bass kernel, 8-way sharded.

Problem: B=4, S=1024, D=1024, H=16 heads of dim 64, fp32.
Sharding: batch (4) x head-half (2) -> 8 cores. Each core computes its
batch's attention output for its 8 heads and the partial output
projection (Wo row-block); the host sums the two half-head partials per
batch and adds the (bv @ Wo + bo) constant.

Per-core design (v2, bf16 datapath; ~131.5us cost-model vs 155.7us for
the f32r v1):
  - All activations/weights ship and compute in bf16 (1 cyc/row on PE at
    any N including the <256-column diagonal tiles, half the DMA/SBUF
    traffic); psum stays f32, output bf16 (summed in f32 on host).
  - Wq/Wk columns are permuted so each 128-row chunk holds 4 heads'
    even (or odd) RoPE coordinates; RoPE runs as 2 wide DVE
    scalar_tensor_tensor ops per psum pair ((e|o)+bias times the packed
    [cos|sin] table) and 2 combines split across DVE and the otherwise
    idle GPSIMD engine.
  - Per-head-contiguous [e;o] score layouts ("pair tiles") are built by
    [32,512] partition-offset copies (bf16 4x on DVE, rest on GPSIMD),
    letting each 128-chunk of transposed scores be 2 matmuls (K=64)
    instead of 4 (K=32) per head pair.
  - Causality: chunk skipping + q0 trimming; the partial diagonal block
    is zeroed after exp by a 2x bf16 DVE multiply with a 0/1 triangle.
  - exp() runs on ACT straight out of PSUM into bf16.
  - V gets a ones-column so softmax denominators fall out of the AV
    matmul (M=65); normalization: DVE reciprocal -> selector-matmul
    broadcast -> one DVE multiply per head into normalized bf16 ctx
    (evictions split ACT/DVE to sit in each engine's lulls).
  - Input DMAs are batched (few, large, rearranged descriptors) and
    ordered so the first rope group and qb0's AV unlock as early as
    possible; the final output tile is split across engines/DMAs to
    shorten the drain tail.
"""

import sys

sys.path.insert(0, "/opt/trn_rl_repo")

import numpy as np
import ml_dtypes

import concourse.bass as bass
import concourse.tile as tile
from concourse import bacc, mybir
from concourse.bass_utils import run_bass_kernel_spmd

P = 128
S = 1024
D = 1024
HD = 64
NH_LOCAL = 8  # heads per core
NB = 2  # S halves for projection psum
QB = 2  # q blocks of 512
KC = 8  # k chunks of 128
F32 = mybir.dt.float32
F32R = mybir.dt.float32r
BF16 = mybir.dt.bfloat16
F8 = mybir.dt.float8e4
DR = mybir.MatmulPerfMode.DoubleRow
# (x_half, w_half) product terms for the hi/lo fp8 split (lo*lo dropped)
FP8_TERMS = ((0, 0), (0, 1), (1, 0))
EXP = mybir.ActivationFunctionType.Exp
IDENT = mybir.ActivationFunctionType.Identity
MULT = mybir.AluOpType.mult
ADD = mybir.AluOpType.add
SUB = mybir.AluOpType.subtract

TRACE = False
LAST_RESULTS = None


def _build_core_program(causal=True):
    nc = bacc.Bacc(None, target_bir_lowering=False)

    # activations/weights ship as fp8 hi/lo plane pairs ([row, 2, col]):
    # hi = fp8(x), lo = fp8(x - hi). Projections run as DoubleRow fp8
    # matmuls over the 3 cross terms (hi*hi, hi*lo, lo*hi) at 0.5 cyc/row
    # -- 25% fewer PE cycles than bf16 at bf16-level precision.
    xqT = nc.declare_dram_parameter("xqT", [D, S, 2], F8, isOutput=False)
    xkT = nc.declare_dram_parameter("xkT", [D, S, 2], F8, isOutput=False)
    xvT = nc.declare_dram_parameter("xvT", [D, S, 2], F8, isOutput=False)
    wq = nc.declare_dram_parameter("wq", [D, 512, 2], F8, isOutput=False)
    wk = nc.declare_dram_parameter("wk", [D, 512, 2], F8, isOutput=False)
    wv = nc.declare_dram_parameter("wv", [D, 512, 2], F8, isOutput=False)
    wo = nc.declare_dram_parameter("wo", [512, D], BF16, isOutput=False)
    bqkp = nc.declare_dram_parameter("bqkp", [P, 8], F32, isOutput=False)
    # cos | sin rope tables side by side
    cs = nc.declare_dram_parameter("cs", [P, 2 * S], BF16, isOutput=False)
    # tri01[k, q] = 1 where q >= k else 0 (causal keep-mask for the
    # diagonal block), next to a PxP identity
    mi = nc.declare_dram_parameter("mi", [P, 2 * P], BF16, isOutput=False)
    selp = nc.declare_dram_parameter("selp", [2, P], BF16, isOutput=False)
    outp = nc.declare_dram_parameter("outp", [S, D], BF16, isOutput=True)

    with tile.TileContext(nc) as tc:
        with (
            tc.tile_pool(name="const", bufs=1) as cpool,
            tc.tile_pool(name="xt", bufs=6) as xtpool,
            tc.tile_pool(name="w", bufs=6) as wpool,
            tc.tile_pool(name="rot", bufs=1) as rotpool,
            tc.tile_pool(name="pair", bufs=1) as pairpool,
            tc.tile_pool(name="vsb", bufs=1) as vpool,
            tc.tile_pool(name="tmp", bufs=3) as tmppool,
            tc.tile_pool(name="es", bufs=16) as espool,
            tc.tile_pool(name="ctxe", bufs=4) as ctxepool,
            tc.tile_pool(name="ctx", bufs=3) as ctxpool,
            tc.tile_pool(name="osb", bufs=8) as opool,
            tc.tile_pool(name="dr", bufs=4) as drpool,
            tc.tile_pool(name="pj", bufs=2, space="PSUM") as pjpool,
            tc.tile_pool(name="sc", bufs=2, space="PSUM") as scpool,
            tc.tile_pool(name="cx", bufs=2, space="PSUM") as cxpool,
        ):
            # ---- constants ----
            cs_sb = cpool.tile([P, 2, S], BF16, tag="cs")
            cos_sb = cs_sb[:, 0, :]
            sin_sb = cs_sb[:, 1, :]
            mi_sb = cpool.tile([P, 2, P], BF16, tag="mi")
            tri_sb = mi_sb[:, 0, :]
            ident_sb = mi_sb[:, 1, :]
            bqk_sb = cpool.tile([P, 8], F32, tag="bqk")
            bq_sb = bqk_sb[:, 0:4]
            bk_sb = bqk_sb[:, 4:8]
            wo_sb = cpool.tile([P, 4, D], BF16, tag="wo")
            sel_sb = cpool.tile([2, P], BF16, tag="sel")
            ones8_sb = cpool.tile([P, NH_LOCAL], BF16, tag="ones8")

            warm = cpool.tile([1, 8], F32, tag="warm")
            nc.gpsimd.memset(warm[:], 0.0)
            nc.scalar.activation(warm[0:1, 4:8], warm[0:1, 0:4], EXP)

            def emit_consts():
                # only what the first rope group needs; the mask/selector
                # consts follow after k's x-loads
                nc.sync.dma_start(cs_sb[:], cs[:].rearrange("p (c s) -> p c s", c=2))
                nc.sync.dma_start(bqk_sb[:], bqkp[:])

            def emit_consts_late():
                nc.sync.dma_start(mi_sb[:], mi[:].rearrange("p (c s) -> p c s", c=2))
                nc.sync.dma_start(sel_sb[:], selp[:])
                # bf16 ones (Memset is avoided): derive from ident
                nc.vector.tensor_scalar(
                    ones8_sb[:], ident_sb[:, 0:NH_LOCAL], 0.0, 1.0, MULT, ADD
                )

            # ---- q/k projections + RoPE + pair-tile build ----
            # rot tiles per (name, G): e and o coordinate chunks [128, S]
            # pair tiles per (name, pair): [h0_e; h0_o; h1_e; h1_o] x [S]
            qpair = {}
            kpair = {}
            xt_cache = {}
            w_cache = {}
            for name in ("q", "k"):
                xt_cache[name] = [
                    xtpool.tile([P, KC // 2, S, 2], F8, tag="xt", name=f"xt_{name}{hf}")
                    for hf in range(2)
                ]
                w_cache[name] = [
                    wpool.tile([P, KC // 2, 512, 2], F8, tag="w", name=f"w_{name}{hf}")
                    for hf in range(2)
                ]

            def emit_w_chunk(name, w, hf, ks2):
                kg = hf * (KC // 2) + 2 * ks2
                nc.sync.dma_start(
                    w_cache[name][hf][:, 2 * ks2 : 2 * ks2 + 2, :, :],
                    w[kg * P : (kg + 2) * P, :, :].rearrange(
                        "(ko ki) f c -> ki ko (f c)", ki=P
                    ),
                )

            def emit_x_chunk(name, xT, nb, hf, ks2):
                kg = hf * (KC // 2) + 2 * ks2
                nc.sync.dma_start(
                    xt_cache[name][hf][
                        :, 2 * ks2 : 2 * ks2 + 2, nb * 512 : (nb + 1) * 512, :
                    ],
                    xT[
                        kg * P : (kg + 2) * P, nb * 512 : (nb + 1) * 512, :
                    ].rearrange("(ko ki) f c -> ki ko (f c)", ki=P),
                )

            def emit_w(name, w):
                for hf in range(2):
                    for ks2 in range(2):
                        emit_w_chunk(name, w, hf, ks2)

            def emit_x_half(name, xT, nb):
                for hf in range(2):
                    for ks2 in range(2):
                        emit_x_chunk(name, xT, nb, hf, ks2)

            def emit_wx_interleaved(name, w, xT, nb):
                # chunk-pair order matches the DR matmul consumption order so
                # the first matmul unblocks after two transfers
                for hf in range(2):
                    for ks2 in range(2):
                        emit_w_chunk(name, w, hf, ks2)
                        emit_x_chunk(name, xT, nb, hf, ks2)

            # nb0 halves + just-in-time consts feed the first rope groups;
            # nb1 halves follow after v's first column half
            emit_wx_interleaved("q", wq, xqT, 0)
            emit_consts()
            emit_wx_interleaved("k", wk, xkT, 0)

            # emit in an order that unlocks qb0 attention as early as
            # possible: (q,G0,nb0),(k,G0,nb0),(q,G1,nb0),(k,G1,nb0), then
            # the nb1 halves.
            def rope_group(name, G, nb):
                b_sb = bq_sb if name == "q" else bk_sb
                xt_h = xt_cache[name]
                w_h = w_cache[name]
                rots = rot_tiles[name]
                rot_e, rot_o = rots[G]
                ce, co = 2 * G, 2 * G + 1
                sl = slice(nb * 512, (nb + 1) * 512)
                ps_e_t = scpool.tile([P, 2, 512], F32, tag="sc", name=f"pse_{name}{G}{nb}")
                ps_e = ps_e_t[:, 0, :]
                ps_o = cxpool.tile([P, 512], F32, tag="cx", name=f"pso_{name}{G}{nb}")
                for ps, c in ((ps_e, ce), (ps_o, co)):
                    idx = 0
                    for hf in range(2):
                        for ks2 in range(2):
                            for xh, wh in FP8_TERMS:
                                nc.tensor.matmul(
                                    ps[:],
                                    w_h[hf][
                                        :, 2 * ks2 : 2 * ks2 + 2, c * P : (c + 1) * P, wh
                                    ],
                                    xt_h[hf][:, 2 * ks2 : 2 * ks2 + 2, sl, xh],
                                    start=(idx == 0),
                                    stop=(idx == 11),
                                    perf_mode=DR,
                                )
                                idx += 1
                # RoPE: rot_e = (e+be)c - (o+bo)s ; rot_o = (e+be)s + (o+bo)c
                # ACT (idle through the projection phase) evicts psum to bf16
                # with the per-partition bias add; the wide c|s products then
                # run as all-SBUF bf16 tensor_tensor on DVE at 4x
                t_eb = tmppool.tile([P, 512], BF16, tag="t1b")
                t_ob = tmppool.tile([P, 512], BF16, tag="t2b")
                nc.scalar.activation(t_eb[:], ps_e[:], IDENT, bias=b_sb[:, ce : ce + 1])
                nc.scalar.activation(t_ob[:], ps_o[:], IDENT, bias=b_sb[:, co : co + 1])
                t_e2 = tmppool.tile([P, 2, 512], BF16, tag="t1")
                t_o2 = tmppool.tile([P, 2, 512], BF16, tag="t2")
                nc.vector.tensor_tensor(
                    t_e2[:],
                    t_eb[:, None, :].to_broadcast((P, 2, 512)),
                    cs_sb[:, :, sl],
                    MULT,
                )
                nc.vector.tensor_tensor(
                    t_o2[:],
                    t_ob[:, None, :].to_broadcast((P, 2, 512)),
                    cs_sb[:, :, sl],
                    MULT,
                )
                # combines split across DVE and the (otherwise idle) GPSIMD
                # engine so both rot halves finish in parallel
                nc.vector.tensor_tensor(rot_e[:, sl], t_e2[:, 0, :], t_o2[:, 1, :], SUB)
                nc.gpsimd.tensor_tensor(rot_o[:, sl], t_e2[:, 1, :], t_o2[:, 0, :], ADD)
                # partition-move into per-head-contiguous pair tiles: bf16
                # SBUF->SBUF copies run at 4x on DVE; o-side copies go to the
                # GPSIMD engine to halve the chain latency
                pairs = qpair if name == "q" else kpair
                for hp in range(2):
                    pt = pairs[2 * G + hp]
                    for ii in range(2):
                        i = 2 * hp + ii
                        nc.vector.tensor_copy(
                            pt[64 * ii : 64 * ii + 32, sl],
                            rot_e[32 * i : 32 * i + 32, sl],
                        )
                        nc.gpsimd.tensor_copy(
                            pt[64 * ii + 32 : 64 * ii + 64, sl],
                            rot_o[32 * i : 32 * i + 32, sl],
                        )

            rot_tiles = {"q": {}, "k": {}}
            for name in ("q", "k"):
                for G in range(2):
                    rot_tiles[name][G] = (
                        rotpool.tile([P, S], BF16, tag=f"{name}re{G}", name=f"{name}re{G}"),
                        rotpool.tile([P, S], BF16, tag=f"{name}ro{G}", name=f"{name}ro{G}"),
                    )
            for pr in range(4):
                qpair[pr] = pairpool.tile([P, S], BF16, tag=f"qp{pr}", name=f"qp{pr}")
                kpair[pr] = pairpool.tile([P, S], BF16, tag=f"kp{pr}", name=f"kp{pr}")

            rope_group("q", 0, 0)
            rope_group("k", 0, 0)
            rope_group("q", 1, 0)
            rope_group("k", 1, 0)

            # ---- v projection (natural layout + ones column) ----
            # v x loaded by position-column halves: chunks 0..3 only read
            # columns [0:512]
            xt_h = []
            w_h = []
            for hf in range(2):
                xt_sb = xtpool.tile([P, KC // 2, S, 2], F8, tag="xt", name=f"xt_v{hf}")
                xt_h.append(xt_sb)
                w_sb = wpool.tile([P, KC // 2, 512, 2], F8, tag="w", name=f"w_v{hf}")
                kg = hf * (KC // 2)
                nc.sync.dma_start(
                    w_sb[:],
                    wv[kg * P : (kg + 4) * P, :, :].rearrange(
                        "(ko ki) f c -> ki ko (f c)", ki=P
                    ),
                )
                w_h.append(w_sb)
            emit_consts_late()

            def emit_xv_half(ch):
                for hf in range(2):
                    for ks2 in range(2):
                        kg = hf * (KC // 2) + 2 * ks2
                        nc.sync.dma_start(
                            xt_h[hf][
                                :, 2 * ks2 : 2 * ks2 + 2, ch * 512 : (ch + 1) * 512, :
                            ],
                            xvT[
                                kg * P : (kg + 2) * P, ch * 512 : (ch + 1) * 512, :
                            ].rearrange("(ko ki) f c -> ki ko (f c)", ki=P),
                        )

            emit_xv_half(0)
            emit_x_half("q", xqT, 1)
            emit_x_half("k", xkT, 1)
            emit_xv_half(1)
            nc.sync.dma_start(
                wo_sb[:], wo[:].rearrange("(ko ki) f -> ki ko f", ki=P)
            )

            v_sb = []
            v_order = [0, 1, 2, 3, 4, 5, 6, 7]
            v_tiles = {}
            for ki in v_order:
                vt = vpool.tile([P, NH_LOCAL, HD + 1], BF16, tag=f"v{ki}")
                v_tiles[ki] = vt
            def v_chunk(ki):
                vt = v_tiles[ki]
                ps_v = pjpool.tile([P, 512], F32, tag="pj", name=f"psv{ki}")
                idx = 0
                for hf in range(2):
                    for ks2 in range(2):
                        for xh, wh in FP8_TERMS:
                            nc.tensor.matmul(
                                ps_v[:],
                                xt_h[hf][
                                    :, 2 * ks2 : 2 * ks2 + 2, ki * P : (ki + 1) * P, xh
                                ],
                                w_h[hf][:, 2 * ks2 : 2 * ks2 + 2, :, wh],
                                start=(idx == 0),
                                stop=(idx == 11),
                                perf_mode=DR,
                            )
                            idx += 1
                nc.scalar.activation(
                    vt[:, :, 0:HD],
                    ps_v[:].rearrange("p (h d) -> p h d", h=NH_LOCAL),
                    mybir.ActivationFunctionType.Copy,
                    scale=1.0 / 512.0,
                )
                nc.vector.tensor_copy(vt[:, :, HD : HD + 1], ones8_sb[:, :, None])

            # ---- attention (emitted per (qb, pair)) ----
            # ctx pair tiles [128, 512] per (pair, qb): rows 0:64 head 2p,
            # rows 64:128 head 2p+1 (feature-transposed, normalized bf16)
            ctxp = {}
            norm_state = {}

            def attn_pair(qb, pair):
                kmax = 4 * qb + 4 if causal else KC
                qp = qpair[pair]
                kp = kpair[pair]
                ctx_ps = {}
                for ii in range(2):
                    ctx_ps[ii] = cxpool.tile(
                        [P, 512], F32, tag="cx", name=f"cx_{qb}_{2 * pair + ii}"
                    )
                # diagonal chunks (with the extra tri-multiply hop) first,
                # so the pair-closing AV is a short chain-free chunk
                if causal and qb == 1:
                    ki_order = [4, 5, 6, 7, 0, 1, 2, 3]
                else:
                    ki_order = list(range(kmax))
                def emit_scores(ki):
                    ksl = slice(ki * P, (ki + 1) * P)
                    es2 = espool.tile([P, 2, 512], BF16, tag="es")
                    j = ki - 4 * qb if causal else -1
                    q0 = max(0, 128 * j)  # first live q col in this block
                    sc2 = scpool.tile([P, 2, 512], F32, tag="sc")
                    for ii in range(2):
                        nc.tensor.matmul(
                            sc2[:, ii, q0:],
                            kp[64 * ii : 64 * ii + 64, ksl],
                            qp[64 * ii : 64 * ii + 64, qb * 512 + q0 : (qb + 1) * 512],
                            start=True,
                            stop=True,
                        )
                    nc.scalar.activation(es2[:, :, q0:], sc2[:, :, q0:], EXP, scale=0.125)
                    if j >= 0:
                        # zero the above-diagonal block of the exp output
                        # (bf16 all-SBUF tensor_tensor runs at 2x on DVE)
                        nc.vector.tensor_tensor(
                            es2[:, :, 128 * j : 128 * (j + 1)],
                            es2[:, :, 128 * j : 128 * (j + 1)],
                            tri_sb[:, None, :].to_broadcast((P, 2, P)),
                            MULT,
                        )
                    return es2, q0

                def emit_av(idx, ki, es2, q0):
                    for ii in range(2):
                        h = 2 * pair + ii
                        nc.tensor.matmul(
                            ctx_ps[ii][0 : HD + 1, q0:],
                            v_tiles[ki][:, h, :],
                            es2[:, ii, q0:],
                            start=(idx == 0),
                            stop=(idx == kmax - 1),
                        )

                # AV trails the scores by one chunk so the pair's first AV
                # (which waits on the cx-ring free) never heads the PE queue
                pend = None
                for idx, ki in enumerate(ki_order):
                    es2, q0 = emit_scores(ki)
                    if pend is not None:
                        emit_av(*pend)
                    pend = (idx, ki, es2, q0)
                emit_av(*pend)
                # evict + reciprocal free the psum ring; the selector
                # broadcast + batched normalize are emitted LATER (see
                # attn_norm) so the next pair's scores/AV keep PE busy while
                # this pair's recip chain drains on ACT/DVE.
                cp = ctxpool.tile([P, 512], BF16, tag=f"cp{pair}", name=f"cp_{qb}_{pair}")
                ctxp[(pair, qb)] = cp
                drs = []
                # recips first (they feed the selector broadcast), then the
                # evicts; everything stays OFF the exp-saturated ACT queue
                # except the very last pair (where the exp stream has ended)
                for half in range(2):
                    dr_t = drpool.tile(
                        [1, 512], BF16, tag="dr", name=f"dr_{qb}_{2 * pair + half}"
                    )
                    with nc.allow_low_precision(
                        reason="softmax denom reciprocal in bf16 (~4e-3 rel)"
                    ):
                        nc.vector.reciprocal(dr_t[0:1, :], ctx_ps[half][HD : HD + 1, :])
                    drs.append(dr_t)
                for half in range(2):
                    if qb == 1 and pair == 3:
                        nc.scalar.copy(
                            cp[64 * half : 64 * half + 64, :], ctx_ps[half][0:HD, :]
                        )
                    elif half == 0:
                        nc.scalar.copy(
                            cp[0:64, :], ctx_ps[0][0:HD, :]
                        )
                    else:
                        nc.vector.tensor_copy(
                            cp[64:128, :], ctx_ps[1][0:HD, :]
                        )
                norm_state[(qb, pair)] = (cp, drs)

            def attn_norm(qb, pair):
                # broadcast each head's recips into its row-half of one shared
                # psum tile, then one batched in-place normalize multiply
                cp, drs = norm_state.pop((qb, pair))
                ps_b = pjpool.tile([P, 512], F32, tag="pj", name=f"psb_{qb}_{pair}")
                for half in range(2):
                    nc.tensor.matmul(
                        ps_b[64 * half : 64 * half + 64, :],
                        sel_sb[0:1, 0:HD],
                        drs[half][0:1, :],
                        start=True,
                        stop=True,
                    )
                nc.vector.tensor_tensor(cp[:], cp[:], ps_b[:], MULT)

            def out_proj(qb, qis=(0, 1, 2, 3)):
                for qi in qis:
                    o_sb = opool.tile([P, D], BF16, tag="o")
                    last = qb == 1 and qi == 3
                    q0g = (qb * 4 + qi) * P
                    for dh in range(2):
                        ps_o = pjpool.tile([P, 512], F32, tag="pj", name=f"po_{qb}_{qi}_{dh}")
                        for pidx in range(4):
                            nc.tensor.matmul(
                                ps_o[:],
                                ctxp[(pidx, qb)][:, qi * P : (qi + 1) * P],
                                wo_sb[:, pidx, dh * 512 : (dh + 1) * 512],
                                start=(pidx == 0),
                                stop=(pidx == 3),
                            )
                        if last and dh == 0:
                            # the very last output tile: split halves across
                            # DVE/ACT and DMA each half as soon as it lands
                            nc.vector.tensor_copy(
                                o_sb[:, dh * 512 : (dh + 1) * 512], ps_o[:]
                            )
                            nc.sync.dma_start(
                                outp[q0g : q0g + P, 0:512], o_sb[:, 0:512]
                            )
                        elif qb == 0:
                            # qb0's out tiles evict during qb1's exp-saturated
                            # window: keep them off ACT
                            nc.vector.tensor_copy(
                                o_sb[:, dh * 512 : (dh + 1) * 512], ps_o[:]
                            )
                        else:
                            nc.scalar.copy(o_sb[:, dh * 512 : (dh + 1) * 512], ps_o[:])
                    if last:
                        nc.sync.dma_start(
                            outp[q0g : q0g + P, 512:D], o_sb[:, 512:D]
                        )
                    else:
                        nc.sync.dma_start(outp[q0g : q0g + P, :], o_sb[:])

            # v chunks 0..3 unlock qb0's AV; emit attention pairs as soon as
            # their rope/pair/v dependencies exist so the scheduler can
            # overlap them with the remaining projections.
            v_chunk(0)
            v_chunk(1)
            rope_group("q", 0, 1)
            v_chunk(2)
            rope_group("k", 0, 1)
            v_chunk(3)
            attn_pair(0, 0)
            rope_group("q", 1, 1)
            attn_pair(0, 1)
            attn_norm(0, 0)
            v_chunk(4)
            attn_pair(0, 2)
            attn_norm(0, 1)
            rope_group("k", 1, 1)
            v_chunk(5)
            attn_pair(0, 3)
            attn_norm(0, 2)
            v_chunk(6)
            v_chunk(7)
            # qb0's output projection interleaves into qb1's exp-bound
            # attention stream as PE filler
            attn_pair(1, 0)
            attn_norm(0, 3)
            attn_norm(1, 0)
            attn_pair(1, 1)
            out_proj(0, qis=(0, 1))
            attn_pair(1, 2)
            attn_norm(1, 1)
            out_proj(0, qis=(2, 3))
            attn_pair(1, 3)
            attn_norm(1, 2)
            attn_norm(1, 3)
            out_proj(1)

    nc.compile()
    return nc


def _host_prep(query, key, value, Wq, bq, Wk, bk, Wv, bv, Wo, bo):
    """Build the 8 per-core input maps + the shared host-side constants."""
    B = query.shape[0]

    # RoPE tables (matches reference._rope_tables)
    inv_freq = (
        1.0 / (10000.0 ** (np.arange(0, HD, 2, dtype=np.float32) / HD))
    ).astype(np.float32)
    pos = np.arange(S, dtype=np.float32)
    ang = pos[:, None] * inv_freq[None, :]  # [S, 32]
    cos_t = np.cos(ang).astype(np.float32)  # [S, 32]
    sin_t = np.sin(ang).astype(np.float32)
    cosf = np.tile(cos_t.T, (4, 1)) / 512.0  # [128, S], undoes the x*8 w*64
    sinf = np.tile(sin_t.T, (4, 1)) / 512.0  # fp8-range pre-scales
    cs = np.concatenate([cosf, sinf], axis=1).astype(ml_dtypes.bfloat16)

    # additive causal mask for the diagonal block: out[k, q] needs
    # 0 where q >= k else -30; out = maskt.T so maskt[q, k]
    qq, kk = np.meshgrid(np.arange(P), np.arange(P), indexing="ij")
    # tri[k, q] = 1 where q >= k (keep), 0 above the causal diagonal
    tri01 = np.where(qq.T >= kk.T, 1.0, 0.0).astype(np.float32)
    ident = np.eye(P, dtype=np.float32)
    mi = np.concatenate([tri01, ident], axis=1).astype(ml_dtypes.bfloat16)
    selp = np.zeros((2, P), np.float32)
    selp[0, 0:64] = 1.0
    selp[1, 64:128] = 1.0
    selp = selp.astype(ml_dtypes.bfloat16)

    bf = ml_dtypes.bfloat16
    f8 = ml_dtypes.float8_e4m3

    # fp8 e4m3 normals start at 2^-6, so ship x pre-scaled by 8 and W by 64
    # (exact power-of-2 scales) to keep hi AND lo planes out of the
    # subnormal floor; the 1/512 descale folds into the cs table (q/k) and
    # the v-eviction Copy scale on device.
    BX, AW = 8.0, 64.0

    def split8(a, scale):
        # [R, C] float32 -> [R, C, 2] fp8 hi/lo planes
        a = a * np.float32(scale)
        hi = a.astype(f8)
        lo = (a - hi.astype(np.float32)).astype(f8)
        return np.ascontiguousarray(np.stack([hi, lo], axis=-1))

    xq8 = {b: split8(np.ascontiguousarray(query[b].T), BX) for b in range(4)}
    xk8 = {b: split8(np.ascontiguousarray(key[b].T), BX) for b in range(4)}
    xv8 = {b: split8(np.ascontiguousarray(value[b].T), BX) for b in range(4)}

    in_maps = []
    for c in range(8):
        b, g = c // 2, c % 2
        perm = np.concatenate(
            [
                (g * 8 + G * 4 + i) * HD + eo + 2 * np.arange(32)
                for G in range(2)
                for eo in range(2)
                for i in range(4)
            ]
        )
        wq_c = split8(Wq[:, perm], AW)
        bq_c = (bq[perm] * BX * AW).astype(np.float32).reshape(4, P).T
        wk_c = split8(Wk[:, perm], AW)
        bk_c = (bk[perm] * BX * AW).astype(np.float32).reshape(4, P).T
        bqk_c = np.concatenate([bq_c, bk_c], axis=1).astype(np.float32)
        wv_c = split8(Wv[:, g * 512 : (g + 1) * 512], AW)
        wo_c = Wo[g * 512 : (g + 1) * 512, :].astype(bf)
        in_maps.append(
            {
                "xqT": xq8[b],
                "xkT": xk8[b],
                "xvT": xv8[b],
                "wq": wq_c,
                "wk": wk_c,
                "wv": wv_c,
                "wo": np.ascontiguousarray(wo_c),
                "bqkp": np.ascontiguousarray(bqk_c),
                "cs": cs,
                "mi": mi,
                "selp": selp,
            }
        )
    extra = (bv.astype(np.float32) @ Wo.astype(np.float32) + bo).astype(np.float32)
    return in_maps, extra


_CACHED = {}


def kernel(query, key, value, mask, Wq, bq, Wk, bk, Wv, bv, Wo, bo):
    global LAST_RESULTS
    query = np.asarray(query, dtype=np.float32)
    key = np.asarray(key, dtype=np.float32)
    value = np.asarray(value, dtype=np.float32)
    Wq, bq = np.asarray(Wq, np.float32), np.asarray(bq, np.float32)
    Wk, bk = np.asarray(Wk, np.float32), np.asarray(bk, np.float32)
    Wv, bv = np.asarray(Wv, np.float32), np.asarray(bv, np.float32)
    Wo, bo = np.asarray(Wo, np.float32), np.asarray(bo, np.float32)

    assert query.shape == (4, S, D), f"kernel hardcodes B=4,S=1024,D=1024, got {query.shape}"
    m2 = np.asarray(mask).reshape(S, S)
    tril = np.tril(np.ones((S, S), m2.dtype))
    if np.array_equal(m2, tril):
        causal = True
    elif np.array_equal(m2, np.ones((S, S), m2.dtype)):
        causal = False
    else:
        raise NotImplementedError("kernel supports causal (tril) or all-ones masks")

    in_maps, extra = _host_prep(query, key, value, Wq, bq, Wk, bk, Wv, bv, Wo, bo)
    if causal not in _CACHED:
        _CACHED[causal] = _build_core_program(causal)
    res = run_bass_kernel_spmd(_CACHED[causal], in_maps, list(range(8)), trace=TRACE)
    LAST_RESULTS = res

    B = query.shape[0]
    out = np.empty((B, S, D), dtype=np.float32)
    for b in range(B):
        out[b] = (
            res.results[2 * b]["outp"].astype(np.float32)
            + res.results[2 * b + 1]["outp"].astype(np.float32)
            + extra
        )
    return out



# revision 28
# speedup vs baseline: 1.1209x; 1.0023x over previous
"""Multi-head attention (RoPE, causal) TRN2 Bass kernel, 8-way sharded.

Problem: B=4, S=1024, D=1024, H=16 heads of dim 64, fp32.
Sharding: batch (4) x head-half (2) -> 8 cores. Each core computes its
batch's attention output for its 8 heads and the partial output
projection (Wo row-block); the host sums the two half-head partials per
batch and adds the (bv @ Wo + bo) constant.

Per-core design (v2, bf16 datapath; ~131.5us cost-model vs 155.7us for
the f32r v1):
  - All activations/weights ship and compute in bf16 (1 cyc/row on PE at
    any N including the <256-column diagonal tiles, half the DMA/SBUF
    traffic); psum stays f32, output bf16 (summed in f32 on host).
  - Wq/Wk columns are permuted so each 128-row chunk holds 4 heads'
    even (or odd) RoPE coordinates; RoPE runs as 2 wide DVE
    scalar_tensor_tensor ops per psum pair ((e|o)+bias times the packed
    [cos|sin] table) and 2 combines split across DVE and the otherwise
    idle GPSIMD engine.
  - Per-head-contiguous [e;o] score layouts ("pair tiles") are built by
    [32,512] partition-offset copies (bf16 4x on DVE, rest on GPSIMD),
    letting each 128-chunk of transposed scores be 2 matmuls (K=64)
    instead of 4 (K=32) per head pair.
  - Causality: chunk skipping + q0 trimming; the partial diagonal block
    is zeroed after exp by a 2x bf16 DVE multiply with a 0/1 triangle.
  - exp() runs on ACT straight out of PSUM into bf16.
  - V gets a ones-column so softmax denominators fall out of the AV
    matmul (M=65); normalization: DVE reciprocal -> selector-matmul
    broadcast -> one DVE multiply per head into normalized bf16 ctx
    (evictions split ACT/DVE to sit in each engine's lulls).
  - Input DMAs are batched (few, large, rearranged descriptors) and
    ordered so the first rope group and qb0's AV unlock as early as
    possible; the final output tile is split across engines/DMAs to
    shorten the drain tail.
"""

import sys

sys.path.insert(0, "/opt/trn_rl_repo")

import numpy as np
import ml_dtypes

import concourse.bass as bass
import concourse.tile as tile
from concourse import bacc, mybir
from concourse.bass_utils import run_bass_kernel_spmd

P = 128
S = 1024
D = 1024
HD = 64
NH_LOCAL = 8  # heads per core
NB = 2  # S halves for projection psum
QB = 2  # q blocks of 512
KC = 8  # k chunks of 128
F32 = mybir.dt.float32
F32R = mybir.dt.float32r
BF16 = mybir.dt.bfloat16
F8 = mybir.dt.float8e4
DR = mybir.MatmulPerfMode.DoubleRow
# (x_half, w_half) product terms for the hi/lo fp8 split (lo*lo dropped)
FP8_TERMS = ((0, 0), (0, 1), (1, 0))
EXP = mybir.ActivationFunctionType.Exp
IDENT = mybir.ActivationFunctionType.Identity
MULT = mybir.AluOpType.mult
ADD = mybir.AluOpType.add
SUB = mybir.AluOpType.subtract

TRACE = False
LAST_RESULTS = None


def _build_core_program(causal=True):
    nc = bacc.Bacc(None, target_bir_lowering=False)

    # activations/weights ship as fp8 hi/lo plane pairs ([row, 2, col]):
    # hi = fp8(x), lo = fp8(x - hi). Projections run as DoubleRow fp8
    # matmuls over the 3 cross terms (hi*hi, hi*lo, lo*hi) at 0.5 cyc/row
    # -- 25% fewer PE cycles than bf16 at bf16-level precision.
    xqT = nc.declare_dram_parameter("xqT", [D, S, 2], F8, isOutput=False)
    xkT = nc.declare_dram_parameter("xkT", [D, S, 2], F8, isOutput=False)
    xvT = nc.declare_dram_parameter("xvT", [D, S, 2], F8, isOutput=False)
    wq = nc.declare_dram_parameter("wq", [D, 512, 2], F8, isOutput=False)
    wk = nc.declare_dram_parameter("wk", [D, 512, 2], F8, isOutput=False)
    wv = nc.declare_dram_parameter("wv", [D, 512, 2], F8, isOutput=False)
    wo = nc.declare_dram_parameter("wo", [512, D], BF16, isOutput=False)
    bqkp = nc.declare_dram_parameter("bqkp", [P, 8], F32, isOutput=False)
    # cos | sin rope tables side by side
    cs = nc.declare_dram_parameter("cs", [P, 2 * S], BF16, isOutput=False)
    # tri01[k, q] = 1 where q >= k else 0 (causal keep-mask for the
    # diagonal block), next to a PxP identity
    mi = nc.declare_dram_parameter("mi", [P, 2 * P], BF16, isOutput=False)
    selp = nc.declare_dram_parameter("selp", [2, P], BF16, isOutput=False)
    outp = nc.declare_dram_parameter("outp", [S, D], BF16, isOutput=True)

    with tile.TileContext(nc) as tc:
        with (
            tc.tile_pool(name="const", bufs=1) as cpool,
            tc.tile_pool(name="xt", bufs=6) as xtpool,
            tc.tile_pool(name="w", bufs=6) as wpool,
            tc.tile_pool(name="rot", bufs=1) as rotpool,
            tc.tile_pool(name="pair", bufs=1) as pairpool,
            tc.tile_pool(name="vsb", bufs=1) as vpool,
            tc.tile_pool(name="tmp", bufs=3) as tmppool,
            tc.tile_pool(name="es", bufs=16) as espool,
            tc.tile_pool(name="ctxe", bufs=4) as ctxepool,
            tc.tile_pool(name="ctx", bufs=3) as ctxpool,
            tc.tile_pool(name="osb", bufs=8) as opool,
            tc.tile_pool(name="dr", bufs=4) as drpool,
            tc.tile_pool(name="pj", bufs=2, space="PSUM") as pjpool,
            tc.tile_pool(name="sc", bufs=2, space="PSUM") as scpool,
            tc.tile_pool(name="cx", bufs=2, space="PSUM") as cxpool,
        ):
            # ---- constants ----
            cs_sb = cpool.tile([P, 2, S], BF16, tag="cs")
            cos_sb = cs_sb[:, 0, :]
            sin_sb = cs_sb[:, 1, :]
            mi_sb = cpool.tile([P, 2, P], BF16, tag="mi")
            tri_sb = mi_sb[:, 0, :]
            ident_sb = mi_sb[:, 1, :]
            bqk_sb = cpool.tile([P, 8], F32, tag="bqk")
            bq_sb = bqk_sb[:, 0:4]
            bk_sb = bqk_sb[:, 4:8]
            wo_sb = cpool.tile([P, 4, D], BF16, tag="wo")
            sel_sb = cpool.tile([2, P], BF16, tag="sel")
            ones8_sb = cpool.tile([P, NH_LOCAL], BF16, tag="ones8")

            warm = cpool.tile([1, 8], F32, tag="warm")
            nc.gpsimd.memset(warm[:], 0.0)
            nc.scalar.activation(warm[0:1, 4:8], warm[0:1, 0:4], EXP)

            def emit_consts():
                # only what the first rope group needs; the mask/selector
                # consts follow after k's x-loads
                nc.sync.dma_start(cs_sb[:], cs[:].rearrange("p (c s) -> p c s", c=2))
                nc.sync.dma_start(bqk_sb[:], bqkp[:])

            def emit_consts_late():
                nc.sync.dma_start(mi_sb[:], mi[:].rearrange("p (c s) -> p c s", c=2))
                nc.sync.dma_start(sel_sb[:], selp[:])
                # bf16 ones (Memset is avoided): derive from ident
                nc.vector.tensor_scalar(
                    ones8_sb[:], ident_sb[:, 0:NH_LOCAL], 0.0, 1.0, MULT, ADD
                )

            # ---- q/k projections + RoPE + pair-tile build ----
            # rot tiles per (name, G): e and o coordinate chunks [128, S]
            # pair tiles per (name, pair): [h0_e; h0_o; h1_e; h1_o] x [S]
            qpair = {}
            kpair = {}
            xt_cache = {}
            w_cache = {}
            for name in ("q", "k"):
                xt_cache[name] = [
                    xtpool.tile([P, KC // 2, S, 2], F8, tag="xt", name=f"xt_{name}{hf}")
                    for hf in range(2)
                ]
                w_cache[name] = [
                    wpool.tile([P, KC // 2, 512, 2], F8, tag="w", name=f"w_{name}{hf}")
                    for hf in range(2)
                ]

            def emit_w_chunk(name, w, hf, ks2):
                kg = hf * (KC // 2) + 2 * ks2
                nc.sync.dma_start(
                    w_cache[name][hf][:, 2 * ks2 : 2 * ks2 + 2, :, :],
                    w[kg * P : (kg + 2) * P, :, :].rearrange(
                        "(ko ki) f c -> ki ko (f c)", ki=P
                    ),
                )

            def emit_x_chunk(name, xT, nb, hf, ks2):
                kg = hf * (KC // 2) + 2 * ks2
                nc.sync.dma_start(
                    xt_cache[name][hf][
                        :, 2 * ks2 : 2 * ks2 + 2, nb * 512 : (nb + 1) * 512, :
                    ],
                    xT[
                        kg * P : (kg + 2) * P, nb * 512 : (nb + 1) * 512, :
                    ].rearrange("(ko ki) f c -> ki ko (f c)", ki=P),
                )

            def emit_w(name, w):
                for hf in range(2):
                    for ks2 in range(2):
                        emit_w_chunk(name, w, hf, ks2)

            def emit_x_half(name, xT, nb):
                for hf in range(2):
                    for ks2 in range(2):
                        emit_x_chunk(name, xT, nb, hf, ks2)

            def emit_wx_interleaved(name, w, xT, nb):
                # chunk-pair order matches the DR matmul consumption order so
                # the first matmul unblocks after two transfers
                for hf in range(2):
                    for ks2 in range(2):
                        emit_w_chunk(name, w, hf, ks2)
                        emit_x_chunk(name, xT, nb, hf, ks2)

            # nb0 halves + just-in-time consts feed the first rope groups;
            # nb1 halves follow after v's first column half
            emit_wx_interleaved("q", wq, xqT, 0)
            emit_consts()
            emit_wx_interleaved("k", wk, xkT, 0)

            # emit in an order that unlocks qb0 attention as early as
            # possible: (q,G0,nb0),(k,G0,nb0),(q,G1,nb0),(k,G1,nb0), then
            # the nb1 halves.
            def rope_group(name, G, nb):
                b_sb = bq_sb if name == "q" else bk_sb
                xt_h = xt_cache[name]
                w_h = w_cache[name]
                rots = rot_tiles[name]
                rot_e, rot_o = rots[G]
                ce, co = 2 * G, 2 * G + 1
                sl = slice(nb * 512, (nb + 1) * 512)
                ps_e_t = scpool.tile([P, 2, 512], F32, tag="sc", name=f"pse_{name}{G}{nb}")
                ps_e = ps_e_t[:, 0, :]
                ps_o = cxpool.tile([P, 512], F32, tag="cx", name=f"pso_{name}{G}{nb}")
                for ps, c in ((ps_e, ce), (ps_o, co)):
                    idx = 0
                    for hf in range(2):
                        for ks2 in range(2):
                            for xh, wh in FP8_TERMS:
                                nc.tensor.matmul(
                                    ps[:],
                                    w_h[hf][
                                        :, 2 * ks2 : 2 * ks2 + 2, c * P : (c + 1) * P, wh
                                    ],
                                    xt_h[hf][:, 2 * ks2 : 2 * ks2 + 2, sl, xh],
                                    start=(idx == 0),
                                    stop=(idx == 11),
                                    perf_mode=DR,
                                )
                                idx += 1
                # RoPE: rot_e = (e+be)c - (o+bo)s ; rot_o = (e+be)s + (o+bo)c
                # ACT (idle through the projection phase) evicts psum to bf16
                # with the per-partition bias add; the wide c|s products then
                # run as all-SBUF bf16 tensor_tensor on DVE at 4x
                t_eb = tmppool.tile([P, 512], BF16, tag="t1b")
                t_ob = tmppool.tile([P, 512], BF16, tag="t2b")
                nc.scalar.activation(t_eb[:], ps_e[:], IDENT, bias=b_sb[:, ce : ce + 1])
                nc.scalar.activation(t_ob[:], ps_o[:], IDENT, bias=b_sb[:, co : co + 1])
                t_e2 = tmppool.tile([P, 2, 512], BF16, tag="t1")
                t_o2 = tmppool.tile([P, 2, 512], BF16, tag="t2")
                nc.vector.tensor_tensor(
                    t_e2[:],
                    t_eb[:, None, :].to_broadcast((P, 2, 512)),
                    cs_sb[:, :, sl],
                    MULT,
                )
                nc.vector.tensor_tensor(
                    t_o2[:],
                    t_ob[:, None, :].to_broadcast((P, 2, 512)),
                    cs_sb[:, :, sl],
                    MULT,
                )
                # combines split across DVE and the (otherwise idle) GPSIMD
                # engine so both rot halves finish in parallel
                nc.vector.tensor_tensor(rot_e[:, sl], t_e2[:, 0, :], t_o2[:, 1, :], SUB)
                nc.gpsimd.tensor_tensor(rot_o[:, sl], t_e2[:, 1, :], t_o2[:, 0, :], ADD)
                # partition-move into per-head-contiguous pair tiles: bf16
                # SBUF->SBUF copies run at 4x on DVE; o-side copies go to the
                # GPSIMD engine to halve the chain latency
                pairs = qpair if name == "q" else kpair
                for hp in range(2):
                    pt = pairs[2 * G + hp]
                    for ii in range(2):
                        i = 2 * hp + ii
                        nc.vector.tensor_copy(
                            pt[64 * ii : 64 * ii + 32, sl],
                            rot_e[32 * i : 32 * i + 32, sl],
                        )
                        nc.gpsimd.tensor_copy(
                            pt[64 * ii + 32 : 64 * ii + 64, sl],
                            rot_o[32 * i : 32 * i + 32, sl],
                        )

            rot_tiles = {"q": {}, "k": {}}
            for name in ("q", "k"):
                for G in range(2):
                    rot_tiles[name][G] = (
                        rotpool.tile([P, S], BF16, tag=f"{name}re{G}", name=f"{name}re{G}"),
                        rotpool.tile([P, S], BF16, tag=f"{name}ro{G}", name=f"{name}ro{G}"),
                    )
            for pr in range(4):
                qpair[pr] = pairpool.tile([P, S], BF16, tag=f"qp{pr}", name=f"qp{pr}")
                kpair[pr] = pairpool.tile([P, S], BF16, tag=f"kp{pr}", name=f"kp{pr}")

            rope_group("q", 0, 0)
            rope_group("k", 0, 0)
            rope_group("q", 1, 0)
            rope_group("k", 1, 0)

            # ---- v projection (natural layout + ones column) ----
            # v x loaded by position-column halves: chunks 0..3 only read
            # columns [0:512]
            xt_h = []
            w_h = []
            for hf in range(2):
                xt_sb = xtpool.tile([P, KC // 2, S, 2], F8, tag="xt", name=f"xt_v{hf}")
                xt_h.append(xt_sb)
                w_sb = wpool.tile([P, KC // 2, 512, 2], F8, tag="w", name=f"w_v{hf}")
                kg = hf * (KC // 2)
                nc.sync.dma_start(
                    w_sb[:],
                    wv[kg * P : (kg + 4) * P, :, :].rearrange(
                        "(ko ki) f c -> ki ko (f c)", ki=P
                    ),
                )
                w_h.append(w_sb)
            emit_consts_late()

            def emit_xv_half(ch):
                for hf in range(2):
                    for ks2 in range(2):
                        kg = hf * (KC // 2) + 2 * ks2
                        nc.sync.dma_start(
                            xt_h[hf][
                                :, 2 * ks2 : 2 * ks2 + 2, ch * 512 : (ch + 1) * 512, :
                            ],
                            xvT[
                                kg * P : (kg + 2) * P, ch * 512 : (ch + 1) * 512, :
                            ].rearrange("(ko ki) f c -> ki ko (f c)", ki=P),
                        )

            emit_xv_half(0)
            emit_x_half("q", xqT, 1)
            emit_x_half("k", xkT, 1)
            emit_xv_half(1)
            nc.sync.dma_start(
                wo_sb[:], wo[:].rearrange("(ko ki) f -> ki ko f", ki=P)
            )

            v_sb = []
            v_order = [0, 1, 2, 3, 4, 5, 6, 7]
            v_tiles = {}
            for ki in v_order:
                vt = vpool.tile([P, NH_LOCAL, HD + 1], BF16, tag=f"v{ki}")
                v_tiles[ki] = vt
            def v_chunk(ki):
                vt = v_tiles[ki]
                ps_v = pjpool.tile([P, 512], F32, tag="pj", name=f"psv{ki}")
                idx = 0
                for hf in range(2):
                    for ks2 in range(2):
                        for xh, wh in FP8_TERMS:
                            nc.tensor.matmul(
                                ps_v[:],
                                xt_h[hf][
                                    :, 2 * ks2 : 2 * ks2 + 2, ki * P : (ki + 1) * P, xh
                                ],
                                w_h[hf][:, 2 * ks2 : 2 * ks2 + 2, :, wh],
                                start=(idx == 0),
                                stop=(idx == 11),
                                perf_mode=DR,
                            )
                            idx += 1
                nc.scalar.activation(
                    vt[:, :, 0:HD],
                    ps_v[:].rearrange("p (h d) -> p h d", h=NH_LOCAL),
                    mybir.ActivationFunctionType.Copy,
                    scale=1.0 / 512.0,
                )
                nc.vector.tensor_copy(vt[:, :, HD : HD + 1], ones8_sb[:, :, None])

            # ---- attention (emitted per (qb, pair)) ----
            # ctx pair tiles [128, 512] per (pair, qb): rows 0:64 head 2p,
            # rows 64:128 head 2p+1 (feature-transposed, normalized bf16)
            ctxp = {}
            norm_state = {}

            def attn_pair(qb, pair):
                kmax = 4 * qb + 4 if causal else KC
                qp = qpair[pair]
                kp = kpair[pair]
                ctx_ps = {}
                for ii in range(2):
                    ctx_ps[ii] = cxpool.tile(
                        [P, 512], F32, tag="cx", name=f"cx_{qb}_{2 * pair + ii}"
                    )
                # diagonal chunks (with the extra tri-multiply hop) first,
                # so the pair-closing AV is a short chain-free chunk
                if causal and qb == 1:
                    ki_order = [4, 5, 6, 7, 0, 1, 2, 3]
                else:
                    ki_order = list(range(kmax))
                def emit_scores(ki):
                    ksl = slice(ki * P, (ki + 1) * P)
                    es2 = espool.tile([P, 2, 512], BF16, tag="es")
                    j = ki - 4 * qb if causal else -1
                    q0 = max(0, 128 * j)  # first live q col in this block
                    sc2 = scpool.tile([P, 2, 512], F32, tag="sc")
                    for ii in range(2):
                        nc.tensor.matmul(
                            sc2[:, ii, q0:],
                            kp[64 * ii : 64 * ii + 64, ksl],
                            qp[64 * ii : 64 * ii + 64, qb * 512 + q0 : (qb + 1) * 512],
                            start=True,
                            stop=True,
                        )
                    nc.scalar.activation(es2[:, :, q0:], sc2[:, :, q0:], EXP, scale=0.125)
                    if j >= 0:
                        # zero the above-diagonal block of the exp output
                        # (bf16 all-SBUF tensor_tensor runs at 2x on DVE)
                        nc.vector.tensor_tensor(
                            es2[:, :, 128 * j : 128 * (j + 1)],
                            es2[:, :, 128 * j : 128 * (j + 1)],
                            tri_sb[:, None, :].to_broadcast((P, 2, P)),
                            MULT,
                        )
                    return es2, q0

                def emit_av(idx, ki, es2, q0):
                    for ii in range(2):
                        h = 2 * pair + ii
                        nc.tensor.matmul(
                            ctx_ps[ii][0 : HD + 1, q0:],
                            v_tiles[ki][:, h, :],
                            es2[:, ii, q0:],
                            start=(idx == 0),
                            stop=(idx == kmax - 1),
                        )

                # AV trails the scores by one chunk so the pair's first AV
                # (which waits on the cx-ring free) never heads the PE queue
                pend = None
                for idx, ki in enumerate(ki_order):
                    es2, q0 = emit_scores(ki)
                    if pend is not None:
                        emit_av(*pend)
                    pend = (idx, ki, es2, q0)
                emit_av(*pend)
                # evict + reciprocal free the psum ring; the selector
                # broadcast + batched normalize are emitted LATER (see
                # attn_norm) so the next pair's scores/AV keep PE busy while
                # this pair's recip chain drains on ACT/DVE.
                cp = ctxpool.tile([P, 512], BF16, tag=f"cp{pair}", name=f"cp_{qb}_{pair}")
                ctxp[(pair, qb)] = cp
                drs = []
                # recips first (they feed the selector broadcast), then the
                # evicts; everything stays OFF the exp-saturated ACT queue
                # except the very last pair (where the exp stream has ended)
                for half in range(2):
                    dr_t = drpool.tile(
                        [1, 512], BF16, tag="dr", name=f"dr_{qb}_{2 * pair + half}"
                    )
                    with nc.allow_low_precision(
                        reason="softmax denom reciprocal in bf16 (~4e-3 rel)"
                    ):
                        nc.vector.reciprocal(dr_t[0:1, :], ctx_ps[half][HD : HD + 1, :])
                    drs.append(dr_t)
                for half in range(2):
                    if qb == 1 and pair == 3:
                        nc.scalar.copy(
                            cp[64 * half : 64 * half + 64, :], ctx_ps[half][0:HD, :]
                        )
                    else:
                        nc.vector.tensor_copy(
                            cp[64 * half : 64 * half + 64, :], ctx_ps[half][0:HD, :]
                        )
                norm_state[(qb, pair)] = (cp, drs)

            def attn_norm(qb, pair):
                # broadcast each head's recips into its row-half of one shared
                # psum tile, then one batched in-place normalize multiply
                cp, drs = norm_state.pop((qb, pair))
                ps_b = pjpool.tile([P, 512], F32, tag="pj", name=f"psb_{qb}_{pair}")
                for half in range(2):
                    nc.tensor.matmul(
                        ps_b[64 * half : 64 * half + 64, :],
                        sel_sb[0:1, 0:HD],
                        drs[half][0:1, :],
                        start=True,
                        stop=True,
                    )
                nc.vector.tensor_tensor(cp[:], cp[:], ps_b[:], MULT)

            def out_proj(qb, qis=(0, 1, 2, 3)):
                for qi in qis:
                    o_sb = opool.tile([P, D], BF16, tag="o")
                    last = qb == 1 and qi == 3
                    q0g = (qb * 4 + qi) * P
                    for dh in range(2):
                        ps_o = pjpool.tile([P, 512], F32, tag="pj", name=f"po_{qb}_{qi}_{dh}")
                        for pidx in range(4):
                            nc.tensor.matmul(
                                ps_o[:],
                                ctxp[(pidx, qb)][:, qi * P : (qi + 1) * P],
                                wo_sb[:, pidx, dh * 512 : (dh + 1) * 512],
                                start=(pidx == 0),
                                stop=(pidx == 3),
                            )
                        if last and dh == 0:
                            # the very last output tile: split halves across
                            # DVE/ACT and DMA each half as soon as it lands
                            nc.vector.tensor_copy(
                                o_sb[:, dh * 512 : (dh + 1) * 512], ps_o[:]
                            )
                            nc.sync.dma_start(
                                outp[q0g : q0g + P, 0:512], o_sb[:, 0:512]
                            )
                        elif qb == 0:
                            # qb0's out tiles evict during qb1's exp-saturated
                            # window: keep them off ACT
                            nc.vector.tensor_copy(
                                o_sb[:, dh * 512 : (dh + 1) * 512], ps_o[:]
                            )
                        else:
                            nc.scalar.copy(o_sb[:, dh * 512 : (dh + 1) * 512], ps_o[:])
                    if last:
                        nc.sync.dma_start(
                            outp[q0g : q0g + P, 512:D], o_sb[:, 512:D]
                        )
                    else:
                        nc.sync.dma_start(outp[q0g : q0g + P, :], o_sb[:])

            # v chunks 0..3 unlock qb0's AV; emit attention pairs as soon as
            # their rope/pair/v dependencies exist so the scheduler can
            # overlap them with the remaining projections.
            v_chunk(0)
            v_chunk(1)
            rope_group("q", 0, 1)
            v_chunk(2)
            rope_group("k", 0, 1)
            v_chunk(3)
            attn_pair(0, 0)
            rope_group("q", 1, 1)
            attn_pair(0, 1)
            attn_norm(0, 0)
            v_chunk(4)
            attn_pair(0, 2)
            attn_norm(0, 1)
            rope_group("k", 1, 1)
            v_chunk(5)
            attn_pair(0, 3)
            attn_norm(0, 2)
            v_chunk(6)
            v_chunk(7)
            # qb0's output projection interleaves into qb1's exp-bound
            # attention stream as PE filler
            attn_pair(1, 0)
            attn_norm(0, 3)
            attn_norm(1, 0)
            attn_pair(1, 1)
            out_proj(0, qis=(0, 1))
            attn_pair(1, 2)
            attn_norm(1, 1)
            out_proj(0, qis=(2, 3))
            attn_pair(1, 3)
            attn_norm(1, 2)
            attn_norm(1, 3)
            out_proj(1)

    nc.compile()
    return nc


def _host_prep(query, key, value, Wq, bq, Wk, bk, Wv, bv, Wo, bo):
    """Build the 8 per-core input maps + the shared host-side constants."""
    B = query.shape[0]

    # RoPE tables (matches reference._rope_tables)
    inv_freq = (
        1.0 / (10000.0 ** (np.arange(0, HD, 2, dtype=np.float32) / HD))
    ).astype(np.float32)
    pos = np.arange(S, dtype=np.float32)
    ang = pos[:, None] * inv_freq[None, :]  # [S, 32]
    cos_t = np.cos(ang).astype(np.float32)  # [S, 32]
    sin_t = np.sin(ang).astype(np.float32)
    cosf = np.tile(cos_t.T, (4, 1)) / 512.0  # [128, S], undoes the x*8 w*64
    sinf = np.tile(sin_t.T, (4, 1)) / 512.0  # fp8-range pre-scales
    cs = np.concatenate([cosf, sinf], axis=1).astype(ml_dtypes.bfloat16)

    # additive causal mask for the diagonal block: out[k, q] needs
    # 0 where q >= k else -30; out = maskt.T so maskt[q, k]
    qq, kk = np.meshgrid(np.arange(P), np.arange(P), indexing="ij")
    # tri[k, q] = 1 where q >= k (keep), 0 above the causal diagonal
    tri01 = np.where(qq.T >= kk.T, 1.0, 0.0).astype(np.float32)
    ident = np.eye(P, dtype=np.float32)
    mi = np.concatenate([tri01, ident], axis=1).astype(ml_dtypes.bfloat16)
    selp = np.zeros((2, P), np.float32)
    selp[0, 0:64] = 1.0
    selp[1, 64:128] = 1.0
    selp = selp.astype(ml_dtypes.bfloat16)

    bf = ml_dtypes.bfloat16
    f8 = ml_dtypes.float8_e4m3

    # fp8 e4m3 normals start at 2^-6, so ship x pre-scaled by 8 and W by 64
    # (exact power-of-2 scales) to keep hi AND lo planes out of the
    # subnormal floor; the 1/512 descale folds into the cs table (q/k) and
    # the v-eviction Copy scale on device.
    BX, AW = 8.0, 64.0

    def split8(a, scale):
        # [R, C] float32 -> [R, C, 2] fp8 hi/lo planes
        a = a * np.float32(scale)
        hi = a.astype(f8)
        lo = (a - hi.astype(np.float32)).astype(f8)
        return np.ascontiguousarray(np.stack([hi, lo], axis=-1))

    xq8 = {b: split8(np.ascontiguousarray(query[b].T), BX) for b in range(4)}
    xk8 = {b: split8(np.ascontiguousarray(key[b].T), BX) for b in range(4)}
    xv8 = {b: split8(np.ascontiguousarray(value[b].T), BX) for b in range(4)}

    in_maps = []
    for c in range(8):
        b, g = c // 2, c % 2
        perm = np.concatenate(
            [
                (g * 8 + G * 4 + i) * HD + eo + 2 * np.arange(32)
                for G in range(2)
                for eo in range(2)
                for i in range(4)
            ]
        )
        wq_c = split8(Wq[:, perm], AW)
        bq_c = (bq[perm] * BX * AW).astype(np.float32).reshape(4, P).T
        wk_c = split8(Wk[:, perm], AW)
        bk_c = (bk[perm] * BX * AW).astype(np.float32).reshape(4, P).T
        bqk_c = np.concatenate([bq_c, bk_c], axis=1).astype(np.float32)
        wv_c = split8(Wv[:, g * 512 : (g + 1) * 512], AW)
        wo_c = Wo[g * 512 : (g + 1) * 512, :].astype(bf)
        in_maps.append(
            {
                "xqT": xq8[b],
                "xkT": xk8[b],
                "xvT": xv8[b],
                "wq": wq_c,
                "wk": wk_c,
                "wv": wv_c,
                "wo": np.ascontiguousarray(wo_c),
                "bqkp": np.ascontiguousarray(bqk_c),
                "cs": cs,
                "mi": mi,
                "selp": selp,
            }
        )
    extra = (bv.astype(np.float32) @ Wo.astype(np.float32) + bo).astype(np.float32)
    return in_maps, extra


_CACHED = {}


def kernel(query, key, value, mask, Wq, bq, Wk, bk, Wv, bv, Wo, bo):
    global LAST_RESULTS
    query = np.asarray(query, dtype=np.float32)
    key = np.asarray(key, dtype=np.float32)
    value = np.asarray(value, dtype=np.float32)
    Wq, bq = np.asarray(Wq, np.float32), np.asarray(bq, np.float32)
    Wk, bk = np.asarray(Wk, np.float32), np.asarray(bk, np.float32)
    Wv, bv = np.asarray(Wv, np.float32), np.asarray(bv, np.float32)
    Wo, bo = np.asarray(Wo, np.float32), np.asarray(bo, np.float32)

    assert query.shape == (4, S, D), f"kernel hardcodes B=4,S=1024,D=1024, got {query.shape}"
    m2 = np.asarray(mask).reshape(S, S)
    tril = np.tril(np.ones((S, S), m2.dtype))
    if np.array_equal(m2, tril):
        causal = True
    elif np.array_equal(m2, np.ones((S, S), m2.dtype)):
        causal = False
    else:
        raise NotImplementedError("kernel supports causal (tril) or all-ones masks")

    in_maps, extra = _host_prep(query, key, value, Wq, bq, Wk, bk, Wv, bv, Wo, bo)
    if causal not in _CACHED:
        _CACHED[causal] = _build_core_program(causal)
    res = run_bass_kernel_spmd(_CACHED[causal], in_maps, list(range(8)), trace=TRACE)
    LAST_RESULTS = res

    B = query.shape[0]
    out = np.empty((B, S, D), dtype=np.float32)
    for b in range(B):
        out[b] = (
            res.results[2 * b]["outp"].astype(np.float32)
            + res.results[2 * b + 1]["outp"].astype(np.float32)
            + extra
        )
    return out

